# revision 1
# baseline (speedup 1.0000x reference)
"""DRIM layer (distorted Rytov inverse-scattering iteration) on Trainium2.

One Bass/Tile program per core (replicated SPMD on 8 cores):
  P1  Z-matrix build via large-branch Hankel evaluation (upper triangle only;
      Z is complex-symmetric), resident in SBUF as fp32r planes
  P2  block LDL^T elimination, Newton-iterated 128x128 block inverses,
      fp32r tensor-engine matmuls
  P3  back-substitution -> X = Z^-1 [-E_inc | -G]
  P4  total field, RSS power model, data vector
  P5  Rytov H^T rows (4608 x 1664 padded) + H^T d
  P7  Gram H H^T (upper blocks) + Jacobi scaling
  P8  scaled SPD block solve (same Newton machinery, real)
  P9  chi = H^T y, output dchi

Host does input packing / output reshape only.
"""
import math
import os
import numpy as np

import concourse.bass as bass
import concourse.bacc as bacc
import concourse.bass_isa as bass_isa
import concourse.mybir as mybir
import concourse.tile as tile
from concourse.bass_utils import run_bass_kernel_spmd

F32 = mybir.dt.float32
F32R = mybir.dt.float32r
U8 = mybir.dt.uint8
AF = mybir.ActivationFunctionType
ALU = mybir.AluOpType
AXX = mybir.AxisListType.X

M = 48
N = M * M
NB = N // 128               # 18
TX = RX = 40
NL = TX * (RX - 1)          # 1560
LPAD = 1664
LB = LPAD // 128            # 13
RW = 256                    # [0:128]=Re plane, [128:256]=Im plane
CW = 256                    # Z-build column chunk
DOI = 3.0
WL = 0.125
K0 = 2.0 * math.pi / WL
IMP = 120.0 * math.pi
GRID_LEN = DOI / M
GRID_RADIUS = math.sqrt(GRID_LEN ** 2 / math.pi)
NOISE = 1e-6

def _j1s(x):
    t2 = (x / 3.0) ** 2
    return x * (0.5 - 0.56249985*t2 + 0.21093573*t2**2 - 0.03954289*t2**3
                + 0.00443319*t2**4 - 0.00031761*t2**5 + 0.00001109*t2**6)

def _y1s(x):
    t2 = (x / 3.0) ** 2
    p = (-0.6366198 + 0.2212091*t2 + 2.1682709*t2**2 - 1.3164827*t2**3
         + 0.3123951*t2**4 - 0.0400976*t2**5 + 0.0027873*t2**6)
    return ((2.0/math.pi) * x * math.log(0.5*x) * _j1s(x) + p) / x

X0C = K0 * GRID_RADIUS
GRID_AREA = 4.0*math.pi*GRID_RADIUS/(2.0*K0) * _j1s(X0C)
C1 = -IMP * math.pi * GRID_RADIUS / 2.0
C2 = _j1s(X0C)
C3R, C3I = _j1s(X0C), _y1s(X0C)
C1C2 = C1 * C2
ZD_RE = C1 * C3R
ZD_IM_C = C1 * C3I
SA = GRID_AREA * K0 * K0
TWO_PI = 2.0 * math.pi
INV_2PI = 1.0 / TWO_PI
LOG10E20 = 20.0 * math.log10(math.e)
CADD = 10.0 * math.log10(WL * WL / (4.0 * math.pi * IMP) / 1e-3)
C20L = 20.0 / math.log(10.0)

F0C = [0.79788456, -0.00000077, -0.00552740, -0.00009512,
       0.00137237, -0.00072805, 0.00014476]
THC = [-0.78539816, -0.04166397, -0.00003954, 0.00262573,
       -0.00054125, -0.00029333, 0.00013558]
F0CS = [c * (3.0 ** k) * C1C2 for k, c in enumerate(F0C)]
THCS = [c * (3.0 ** k) for k, c in enumerate(THC)]

NEWTON_Z = 22
NEWTON_SPD = 22


def _horner(nc, out_ap, s_ap, coeffs):
    cs = coeffs[::-1]
    nc.vector.tensor_scalar(out=out_ap, in0=s_ap, scalar1=float(cs[0]),
                            scalar2=float(cs[1]), op0=ALU.mult, op1=ALU.add)
    for c in cs[2:]:
        nc.vector.tensor_tensor(out=out_ap, in0=out_ap, in1=s_ap, op=ALU.mult)
        nc.vector.tensor_scalar(out=out_ap, in0=out_ap, scalar1=float(c),
                                scalar2=None, op0=ALU.add)


def _cmm(nc, pool, lhsT, rhs, n=RW):
    P1 = pool.tile([128, n], F32, tag="cmmp1")
    P2 = pool.tile([128, n], F32, tag="cmmp2")
    nc.tensor.matmul(P1[:], lhsT[:, 0:128], rhs, start=True, stop=True)
    nc.tensor.matmul(P2[:], lhsT[:, 128:256], rhs, start=True, stop=True)
    return P1, P2


def _combine_sub(nc, dst, P1, P2):
    nc.vector.tensor_tensor(out=dst[:, 0:256], in0=dst[:, 0:256],
                            in1=P1[:, 0:256], op=ALU.subtract)
    nc.vector.tensor_tensor(out=dst[:, 0:128], in0=dst[:, 0:128],
                            in1=P2[:, 128:256], op=ALU.add)
    nc.vector.tensor_tensor(out=dst[:, 128:256], in0=dst[:, 128:256],
                            in1=P2[:, 0:128], op=ALU.subtract)


def _combine_set(nc, dst, P1, P2):
    nc.vector.tensor_copy(dst[:, 0:256], P1[:, 0:256])
    nc.vector.tensor_tensor(out=dst[:, 0:128], in0=dst[:, 0:128],
                            in1=P2[:, 128:256], op=ALU.subtract)
    nc.vector.tensor_tensor(out=dst[:, 128:256], in0=dst[:, 128:256],
                            in1=P2[:, 0:128], op=ALU.add)


def _newton_scale(nc, work, pmisc, m, tag):
    """1/(colmax * rowmax) of m [128,128] -> [128,1] fp32 AP."""
    ones = work.tile([128, 1], F32, tag=f"nwo_{tag}")
    nc.vector.memset(ones[:], 1.0)
    pc = pmisc.tile([128, 1], F32, tag=f"nwpc_{tag}")
    nc.tensor.matmul(pc[:], m[:], ones[:], start=True, stop=True)
    pr = pmisc.tile([1, 128], F32, tag=f"nwpr_{tag}")
    nc.tensor.matmul(pr[:], ones[:], m[:], start=True, stop=True)
    cs = work.tile([128, 1], F32, tag=f"nwcs_{tag}")
    nc.vector.tensor_copy(cs[:], pc[:])
    rs = work.tile([1, 128], F32, tag=f"nwrs_{tag}")
    nc.vector.tensor_copy(rs[:], pr[:])
    nc.gpsimd.partition_all_reduce(cs[:], cs[:], 128, bass_isa.ReduceOp.max)
    rmax = work.tile([1, 1], F32, tag=f"nwrm_{tag}")
    nc.vector.tensor_reduce(rmax[:], rs[:], axis=AXX, op=ALU.max)
    rmax_b = work.tile([128, 1], F32, tag=f"nwrb_{tag}")
    nc.gpsimd.partition_broadcast(rmax_b[:], rmax[:])
    a = work.tile([128, 1], F32, tag=f"nwa_{tag}")
    nc.vector.tensor_tensor(out=a[:], in0=cs[:], in1=rmax_b[:], op=ALU.mult)
    nc.vector.reciprocal(a[:], a[:])
    return a


def _newton_cplx(nc, work, pmm, pmisc, D, Xout, id_s, iters):
    m = work.tile([128, 128], F32, tag="nw_m")
    m2 = work.tile([128, 128], F32, tag="nw_m2")
    nc.scalar.activation(m[:], D[:, 0:128], AF.Abs)
    nc.scalar.activation(m2[:], D[:, 128:256], AF.Abs)
    nc.vector.tensor_tensor(out=m[:], in0=m[:], in1=m2[:], op=ALU.max)
    a = _newton_scale(nc, work, pmisc, m, "c")
    nc.vector.tensor_scalar(out=Xout[:, 0:128], in0=D[:, 0:128], scalar1=a[:],
                            scalar2=None, op0=ALU.mult)
    nc.vector.tensor_scalar(out=Xout[:, 128:256], in0=D[:, 128:256],
                            scalar1=a[:], scalar2=None, op0=ALU.mult)
    nc.vector.tensor_scalar(out=Xout[:, 128:256], in0=Xout[:, 128:256],
                            scalar1=-1.0, scalar2=None, op0=ALU.mult)
    R = work.tile([128, RW], F32R, tag="nw_R")
    for _ in range(iters):
        P1, P2 = _cmm(nc, pmm, D, Xout[:, 0:RW])
        nc.vector.tensor_tensor(out=R[:, 0:128], in0=id_s[:],
                                in1=P1[:, 0:128], op=ALU.subtract)
        nc.vector.tensor_tensor(out=R[:, 0:128], in0=R[:, 0:128],
                                in1=P2[:, 128:256], op=ALU.add)
        nc.vector.tensor_scalar(out=R[:, 128:256], in0=P1[:, 128:256],
                                scalar1=-1.0, scalar2=None, op0=ALU.mult)
        nc.vector.tensor_tensor(out=R[:, 128:256], in0=R[:, 128:256],
                                in1=P2[:, 0:128], op=ALU.subtract)
        Q1, Q2 = _cmm(nc, pmm, Xout, R[:, 0:RW])
        nc.vector.tensor_tensor(out=Xout[:, 0:256], in0=Xout[:, 0:256],
                                in1=Q1[:, 0:256], op=ALU.add)
        nc.vector.tensor_tensor(out=Xout[:, 0:128], in0=Xout[:, 0:128],
                                in1=Q2[:, 128:256], op=ALU.subtract)
        nc.vector.tensor_tensor(out=Xout[:, 128:256], in0=Xout[:, 128:256],
                                in1=Q2[:, 0:128], op=ALU.add)


def _newton_real(nc, work, pmm, pmisc, D, Xout, id_s, iters):
    m = work.tile([128, 128], F32, tag="nw_m")
    nc.scalar.activation(m[:], D[:], AF.Abs)
    a = _newton_scale(nc, work, pmisc, m, "r")
    nc.vector.tensor_scalar(out=Xout[:], in0=D[:], scalar1=a[:], scalar2=None,
                            op0=ALU.mult)
    R = work.tile([128, 128], F32R, tag="nw_R")
    for _ in range(iters):
        P1 = pmm.tile([128, 128], F32, tag="cmmp1")
        nc.tensor.matmul(P1[:], D[:], Xout[:], start=True, stop=True)
        nc.vector.tensor_tensor(out=R[:], in0=id_s[:], in1=P1[:],
                                op=ALU.subtract)
        Q1 = pmm.tile([128, 128], F32, tag="cmmp2")
        nc.tensor.matmul(Q1[:], Xout[:], R[:], start=True, stop=True)
        nc.vector.tensor_tensor(out=Xout[:], in0=Xout[:], in1=Q1[:], op=ALU.add)


def build_program(link_groups, alpha):
    nc = bacc.Bacc("TRN2", target_bir_lowering=False, num_devices=8)
    din = {}
    def inp(name, shape, dtype=F32):
        din[name] = nc.dram_tensor(name, shape, dtype, kind="ExternalInput")
    inp("geomS", [4, N]); inp("geomR", [4, N]); inp("scat_t", [128, NB])
    inp("bpack", [N, RW]); inp("gscT", [N, 80]); inp("dfpack", [40, 80])
    inp("tpT", [40, RX - 1]); inp("id128", [128, 128]); inp("idu8", [128, 128], U8)
    out_chi = nc.dram_tensor("out_chi", [2 * N], F32, kind="ExternalOutput")
    xdbg = nc.dram_tensor("xdbg", [N, RW], F32, kind="ExternalOutput")
    tfdbg = nc.dram_tensor("tfdbg", [40, 80], F32, kind="ExternalOutput")
    ddbg = nc.dram_tensor("ddbg", [40, RX - 1], F32, kind="ExternalOutput")
    scr = {}
    scr["vdram"] = nc.dram_tensor("vdram", [NB * 128, RW], F32R, kind="Internal")
    scr["utdram"] = nc.dram_tensor("utdram", [N, 2 * N], F32R, kind="Internal")
    scr["htdram"] = nc.dram_tensor("htdram", [2 * N, LPAD], F32, kind="Internal")
    scr["gramdram"] = nc.dram_tensor("gramdram", [LPAD, LPAD], F32, kind="Internal")
    scr["v2dram"] = nc.dram_tensor("v2dram", [LB * 128, 128], F32R, kind="Internal")
    scr["ut2dram"] = nc.dram_tensor("ut2dram", [LPAD, LPAD], F32R, kind="Internal")
    scr["sdram"] = nc.dram_tensor("sdram", [NL], F32, kind="Internal")
    scr["wdram"] = nc.dram_tensor("wdram", [2 * NL], F32, kind="Internal")
    scr["srowdram"] = nc.dram_tensor("srowdram", [LPAD], F32, kind="Internal")
    scr["yrowdram"] = nc.dram_tensor("yrowdram", [LPAD], F32, kind="Internal")

    with tile.TileContext(nc) as tc:
        _body(nc, tc, din, out_chi, xdbg, tfdbg, ddbg, scr, link_groups, alpha)
    nc.compile()
    return nc


def _body(nc, tc, din, out_chi, xdbg, tfdbg, ddbg, scr, link_groups, alpha):
    import contextlib
    ctx = contextlib.ExitStack()
    consts = ctx.enter_context(tc.tile_pool(name="consts", bufs=1))
    id_s = consts.tile([128, 128], F32)
    nc.sync.dma_start(id_s[:], din["id128"][:])
    idr_s = consts.tile([128, 128], F32R)
    nc.vector.tensor_copy(idr_s[:], id_s[:])
    idu_s = consts.tile([128, 128], U8)
    nc.sync.dma_start(idu_s[:], din["idu8"][:])
    scat_s = consts.tile([128, NB], F32)
    nc.sync.dma_start(scat_s[:], din["scat_t"][:])

    zdi_s = consts.tile([128, NB], F32)
    fsc_s = consts.tile([128, NB], F32)
    t0 = consts.tile([128, NB], F32)
    nc.vector.tensor_scalar(out=t0[:], in0=scat_s[:], scalar1=-1.0,
                            scalar2=None, op0=ALU.add)
    nc.vector.reciprocal(t0[:], t0[:])
    nc.vector.tensor_scalar(out=fsc_s[:], in0=t0[:], scalar1=(IMP / K0),
                            scalar2=None, op0=ALU.mult)
    nc.vector.tensor_tensor(out=t0[:], in0=t0[:], in1=scat_s[:], op=ALU.mult)
    nc.vector.tensor_scalar(out=zdi_s[:], in0=t0[:], scalar1=-(IMP / K0),
                            scalar2=ZD_IM_C, op0=ALU.mult, op1=ALU.add)
    zdr_c = consts.tile([128, 1], F32)
    nc.vector.memset(zdr_c[:], float(ZD_RE))

    bf_pool = ctx.enter_context(tc.tile_pool(name="bf", bufs=1))
    BF = [bf_pool.tile([128, RW], F32R, tag=f"bf{i}", name=f"bf{i}") for i in range(NB)]

    with tc.tile_pool(name="tri", bufs=1) as tri:
        ZT = {}
        for i in range(NB):
            for j in range(i, NB):
                ZT[(i, j)] = tri.tile([128, RW], F32R, tag=f"z{i}_{j}", name=f"z{i}_{j}")

        # ---------------- P1: Z build ----------------
        with (
            tc.tile_pool(name="zb_geom", bufs=2) as gpool,
            tc.tile_pool(name="zb_work", bufs=1) as work,
            tc.tile_pool(name="zb_psum", bufs=2, space="PSUM") as pz,
        ):
            for k in range(NB):
                r0 = 128 * k
                gS = gpool.tile([4, 128], F32, tag="gS", name="gS")
                nc.sync.dma_start(gS[:], din["geomS"][:, r0:r0+128])
                j = k
                while j < NB:
                    c0 = 128 * j
                    w = 256 if j + 1 < NB else 128
                    gR = work.tile([4, CW], F32, tag="gR", name="gR")
                    nc.sync.dma_start(gR[:, 0:w], din["geomR"][:, c0:c0+w])
                    # one 128-col block per chunk (CW=256 covers Re|Im writes)
                    pd = pz.tile([128, CW], F32, tag="zb_pd")
                    nc.tensor.matmul(pd[:, 0:w], gS[:], gR[:, 0:w],
                                     start=True, stop=True)
                    dsq = work.tile([128, CW], F32, tag="zb_dsq")
                    nc.vector.tensor_scalar(out=dsq[:, 0:w], in0=pd[:, 0:w],
                                            scalar1=0.002, scalar2=None,
                                            op0=ALU.max)
                    x = work.tile([128, CW], F32, tag="zb_x")
                    nc.scalar.activation(x[:, 0:w], dsq[:, 0:w], AF.Sqrt,
                                         scale=float(K0 * K0))
                    sp = work.tile([128, CW], F32, tag="zb_sp")
                    nc.vector.reciprocal(sp[:, 0:w], x[:, 0:w])
                    f0 = work.tile([128, CW], F32, tag="zb_f0")
                    _horner(nc, f0[:, 0:w], sp[:, 0:w], F0CS)
                    th = work.tile([128, CW], F32, tag="zb_th")
                    _horner(nc, th[:, 0:w], sp[:, 0:w], THCS)
                    nc.vector.tensor_tensor(out=th[:, 0:w], in0=th[:, 0:w],
                                            in1=x[:, 0:w], op=ALU.add)
                    nc.scalar.activation(x[:, 0:w], sp[:, 0:w], AF.Sqrt)
                    nc.vector.tensor_tensor(out=f0[:, 0:w], in0=f0[:, 0:w],
                                            in1=x[:, 0:w], op=ALU.mult)
                    u = work.tile([128, CW], F32, tag="zb_u")
                    nc.vector.tensor_scalar(out=u[:, 0:w], in0=th[:, 0:w],
                                            scalar1=INV_2PI, scalar2=None,
                                            op0=ALU.mult)
                    ki = work.tile([128, CW], mybir.dt.int32, tag="zb_ki")
                    nc.vector.tensor_copy(ki[:, 0:w], u[:, 0:w])
                    mf = work.tile([128, CW], F32, tag="zb_mf")
                    nc.vector.tensor_copy(mf[:, 0:w], ki[:, 0:w])
                    r1 = work.tile([128, CW], F32, tag="zb_r1")
                    nc.vector.tensor_scalar(out=r1[:, 0:w], in0=mf[:, 0:w],
                                            scalar1=-TWO_PI, scalar2=None,
                                            op0=ALU.mult)
                    nc.vector.tensor_tensor(out=r1[:, 0:w], in0=r1[:, 0:w],
                                            in1=th[:, 0:w], op=ALU.add)
                    sinr = work.tile([128, CW], F32, tag="zb_sin")
                    nc.scalar.activation(sinr[:, 0:w], r1[:, 0:w], AF.Sin)
                    nc.vector.tensor_scalar(out=u[:, 0:w], in0=u[:, 0:w],
                                            scalar1=0.25, scalar2=None, op0=ALU.add)
                    nc.vector.tensor_copy(ki[:, 0:w], u[:, 0:w])
                    nc.vector.tensor_copy(mf[:, 0:w], ki[:, 0:w])
                    nc.vector.tensor_scalar(out=mf[:, 0:w], in0=mf[:, 0:w],
                                            scalar1=-TWO_PI,
                                            scalar2=(math.pi / 2.0),
                                            op0=ALU.mult, op1=ALU.add)
                    nc.vector.tensor_tensor(out=mf[:, 0:w], in0=mf[:, 0:w],
                                            in1=th[:, 0:w], op=ALU.add)
                    cosr = work.tile([128, CW], F32, tag="zb_cos")
                    nc.scalar.activation(cosr[:, 0:w], mf[:, 0:w], AF.Sin)
                    nc.vector.tensor_tensor(out=cosr[:, 0:w], in0=cosr[:, 0:w],
                                            in1=f0[:, 0:w], op=ALU.mult)
                    nc.vector.tensor_tensor(out=sinr[:, 0:w], in0=sinr[:, 0:w],
                                            in1=f0[:, 0:w], op=ALU.mult)
                    if j == k:
                        nc.vector.copy_predicated(
                            cosr[:, 0:128], idu_s[:],
                            zdr_c[:].broadcast_to([128, 128]))
                        nc.vector.copy_predicated(
                            sinr[:, 0:128], idu_s[:],
                            zdi_s[:, k:k+1].broadcast_to([128, 128]))
                    for b in range(w // 128):
                        nc.vector.tensor_copy(ZT[(k, j + b)][:, 0:128],
                                              cosr[:, 128*b:128*b+128])
                        nc.vector.tensor_copy(ZT[(k, j + b)][:, 128:256],
                                              sinr[:, 128*b:128*b+128])
                    j += w // 128

        # ---------------- P2: block LDL^T ----------------
        with (
            tc.tile_pool(name="lu_work", bufs=2) as work,
            tc.tile_pool(name="lu_pmm", bufs=2, space="PSUM") as pmm,
            tc.tile_pool(name="lu_pmisc", bufs=1, space="PSUM") as pmisc,
        ):
            ldtmp0 = work.tile([128, RW], F32, tag="ldtmp")
            for i in range(NB):
                nc.sync.dma_start(ldtmp0[:], din["bpack"][128*i:128*(i+1), :])
                nc.vector.tensor_copy(BF[i][:], ldtmp0[:])
                ldtmp0 = work.tile([128, RW], F32, tag="ldtmp")
            for k in range(NB):
                V = work.tile([128, RW], F32R, tag="lu_V")
                _newton_cplx(nc, work, pmm, pmisc, ZT[(k, k)], V, id_s, NEWTON_Z)
                nc.sync.dma_start(scr["vdram"][128*k:128*(k+1), :], V[:])
                for i in range(k + 1, NB):
                    ptr = pmisc.tile([128, 128], F32R, tag="lu_ptr")
                    nc.tensor.transpose(ptr[:], ZT[(k, i)][:, 0:128], idr_s[:])
                    utt = work.tile([128, RW], F32R, tag="lu_utt")
                    nc.vector.tensor_copy(utt[:, 0:128], ptr[:])
                    pti = pmisc.tile([128, 128], F32R, tag="lu_pti")
                    nc.tensor.transpose(pti[:], ZT[(k, i)][:, 128:256], idr_s[:])
                    nc.vector.tensor_copy(utt[:, 128:256], pti[:])
                    nc.sync.dma_start(
                        scr["utdram"][128*i:128*(i+1), 256*k:256*(k+1)], utt[:])
                for i in range(k + 1, NB):
                    P1, P2 = _cmm(nc, pmm, V, ZT[(k, i)][:, 0:RW])
                    LT = work.tile([128, RW], F32R, tag="lu_LT")
                    _combine_set(nc, LT, P1, P2)
                    LTn = work.tile([128, 128], F32R, tag="lu_LTn")
                    nc.vector.tensor_scalar(out=LTn[:], in0=LT[:, 128:256],
                                            scalar1=-1.0, scalar2=None,
                                            op0=ALU.mult)

                    def upd(rhs, dst):
                        P = pmm.tile([128, RW], F32, tag="cmmp1", name="updP")
                        nc.tensor.matmul(P[:, 0:128], LT[:, 0:128],
                                         rhs[:, 0:128], start=True, stop=False)
                        nc.tensor.matmul(P[:, 0:128], LTn[:],
                                         rhs[:, 128:256], start=False, stop=True)
                        nc.tensor.matmul(P[:, 128:256], LT[:, 0:128],
                                         rhs[:, 128:256], start=True, stop=False)
                        nc.tensor.matmul(P[:, 128:256], LT[:, 128:256],
                                         rhs[:, 0:128], start=False, stop=True)
                        nc.vector.tensor_tensor(out=dst[:, 0:256],
                                                in0=dst[:, 0:256],
                                                in1=P[:, 0:256],
                                                op=ALU.subtract)

                    upd(BF[k], BF[i])
                    for j in range(i, NB):
                        upd(ZT[(k, j)], ZT[(i, j)])

    # ---------------- P3: back-substitution ----------------
    with (
        tc.tile_pool(name="bs_work", bufs=3) as work,
        tc.tile_pool(name="bs_pacc", bufs=1, space="PSUM") as pacc,
        tc.tile_pool(name="bs_pmm", bufs=2, space="PSUM") as pmm,
    ):
        for k in range(NB - 1, -1, -1):
            W = work.tile([128, RW], F32R, tag="bs_W")
            nc.vector.tensor_copy(W[:], BF[k][:])
            if k < NB - 1:
                P1 = pacc.tile([128, RW], F32, tag="bs_p1")
                P2 = pacc.tile([128, RW], F32, tag="bs_p2")
                for idx, j in enumerate(range(k + 1, NB)):
                    utt = work.tile([128, RW], F32R, tag="bs_utt")
                    nc.sync.dma_start(
                        utt[:], scr["utdram"][128*j:128*(j+1), 256*k:256*(k+1)])
                    st = (idx == 0); sp_ = (j == NB - 1)
                    nc.tensor.matmul(P1[:], utt[:, 0:128], BF[j][:, 0:RW],
                                     start=st, stop=sp_)
                    nc.tensor.matmul(P2[:], utt[:, 128:256], BF[j][:, 0:RW],
                                     start=st, stop=sp_)
                _combine_sub(nc, W, P1, P2)
            Vk = work.tile([128, RW], F32R, tag="bs_V")
            nc.sync.dma_start(Vk[:], scr["vdram"][128*k:128*(k+1), :])
            P1, P2 = _cmm(nc, pmm, Vk, W[:, 0:RW])
            _combine_set(nc, BF[k], P1, P2)
            nc.sync.dma_start(xdbg[128*k:128*(k+1), :], BF[k][:].bitcast(F32))

    # ---------------- P4: tf + data vector ----------------
    late = ctx.enter_context(tc.tile_pool(name="late", bufs=1))
    dvec = late.tile([128, LB], F32)
    drep = late.tile([128, LPAD], F32)
    wrep_r = late.tile([128, NL], F32)
    wrep_i = late.tile([128, NL], F32)
    with (
        tc.tile_pool(name="p4_work", bufs=2) as work,
        tc.tile_pool(name="p4_pacc", bufs=1, space="PSUM") as pacc,
        tc.tile_pool(name="p4_pmisc", bufs=1, space="PSUM") as pmisc,
    ):
        Ptf1 = pacc.tile([40, RW], F32, tag="tf_p1")
        Ptf2 = pacc.tile([40, RW], F32, tag="tf_p2")
        for i in range(NB):
            gt = work.tile([128, 80], F32, tag="tf_g")
            nc.sync.dma_start(gt[:], din["gscT"][128*i:128*(i+1), :])
            gtr = work.tile([128, 80], F32R, tag="tf_gr")
            nc.vector.tensor_copy(gtr[:], gt[:])
            st = (i == 0); sp_ = (i == NB - 1)
            nc.tensor.matmul(Ptf1[:], gtr[:, 0:40], BF[i][:, 0:RW],
                             start=st, stop=sp_)
            nc.tensor.matmul(Ptf2[:], gtr[:, 40:80], BF[i][:, 0:RW],
                             start=st, stop=sp_)
        df = work.tile([40, 80], F32, tag="tf_df")
        nc.sync.dma_start(df[:], din["dfpack"][:])
        tfr = work.tile([40, 40], F32, tag="tfr")
        tfi = work.tile([40, 40], F32, tag="tfi")
        nc.vector.tensor_tensor(out=tfr[:], in0=df[:, 0:40],
                                in1=Ptf1[:, 0:40], op=ALU.add)
        nc.vector.tensor_tensor(out=tfr[:], in0=tfr[:],
                                in1=Ptf2[:, 128:168], op=ALU.subtract)
        nc.vector.tensor_tensor(out=tfi[:], in0=df[:, 40:80],
                                in1=Ptf1[:, 128:168], op=ALU.add)
        nc.vector.tensor_tensor(out=tfi[:], in0=tfi[:],
                                in1=Ptf2[:, 0:40], op=ALU.add)
        tfd = work.tile([40, 80], F32, tag="tf_out")
        nc.vector.tensor_copy(tfd[:, 0:40], tfr[:])
        nc.vector.tensor_copy(tfd[:, 40:80], tfi[:])
        nc.sync.dma_start(tfdbg[:], tfd[:])

        pw = work.tile([40, 40], F32, tag="pw")
        nc.vector.tensor_tensor(out=pw[:], in0=tfr[:], in1=tfr[:], op=ALU.mult)
        t1 = work.tile([40, 40], F32, tag="pw_t")
        nc.vector.tensor_tensor(out=t1[:], in0=tfi[:], in1=tfi[:], op=ALU.mult)
        nc.vector.tensor_tensor(out=pw[:], in0=pw[:], in1=t1[:], op=ALU.add)
        amp = work.tile([40, 40], F32, tag="amp")
        nc.scalar.activation(amp[:], pw[:], AF.Sqrt)
        nc.vector.tensor_scalar(out=amp[:], in0=amp[:], scalar1=NOISE,
                                scalar2=None, op0=ALU.add)
        nc.scalar.activation(amp[:], amp[:], AF.Ln)
        tpi = work.tile([40, 40], F32, tag="tpi")
        nc.vector.tensor_scalar(out=tpi[:], in0=amp[:], scalar1=C20L,
                                scalar2=CADD, op0=ALU.mult, op1=ALU.add)
        rec = work.tile([40, 40], F32, tag="rec")
        nc.vector.reciprocal(rec[:], pw[:])
        wr = work.tile([40, 40], F32, tag="wr")
        nc.vector.tensor_tensor(out=wr[:], in0=tfr[:], in1=rec[:], op=ALU.mult)
        nc.vector.tensor_scalar(out=wr[:], in0=wr[:], scalar1=SA, scalar2=None,
                                op0=ALU.mult)
        wi = work.tile([40, 40], F32, tag="wi")
        nc.vector.tensor_tensor(out=wi[:], in0=tfi[:], in1=rec[:], op=ALU.mult)
        nc.vector.tensor_scalar(out=wi[:], in0=wi[:], scalar1=-SA, scalar2=None,
                                op0=ALU.mult)

        def t40(src, name):
            pt = pmisc.tile([40, 40], F32, tag=f"t40p_{name}")
            nc.tensor.matmul(pt[:], src[:], id_s[0:40, 0:40], start=True,
                             stop=True)
            d = work.tile([40, 40], F32, tag=f"t40_{name}")
            nc.vector.tensor_copy(d[:], pt[:])
            return d
        tpiT = t40(tpi, "tpi"); wrT = t40(wr, "wr"); wiT = t40(wi, "wi")

        pack = work.tile([40, 120], F32, tag="pack")
        nc.vector.tensor_copy(pack[:, 0:40], tpiT[:])
        nc.vector.tensor_copy(pack[:, 40:80], wrT[:])
        nc.vector.tensor_copy(pack[:, 80:120], wiT[:])
        kept3 = work.tile([1, 3 * NL], F32, tag="kept3")
        pack3d = pack[:].rearrange("p (a b) -> p a b", a=3)
        kept3d = kept3[:].rearrange("p (a b) -> p a b", a=3)
        for (t, rs_list) in link_groups:
            o = _GBASE[t]
            for (s0, ln) in _contig_segments(rs_list):
                nc.sync.dma_start(kept3d[0:1, :, o:o+ln],
                                  pack3d[t:t+1, :, s0:s0+ln])
                o += ln
        # data = (tpT - tpi_kept)/LOG10E20 on the packed row
        tprow = work.tile([1, NL], F32, tag="tprow")
        nc.sync.dma_start(tprow[:], bass.AP(din["tpT"], 0, [[1, NL]]))
        nc.vector.tensor_tensor(out=kept3[0:1, 0:NL], in0=tprow[:],
                                in1=kept3[0:1, 0:NL], op=ALU.subtract)
        nc.vector.tensor_scalar(out=kept3[0:1, 0:NL], in0=kept3[0:1, 0:NL],
                                scalar1=1.0 / LOG10E20, scalar2=None,
                                op0=ALU.mult)
        nc.sync.dma_start(bass.AP(ddbg, 0, [[1, NL]]), kept3[0:1, 0:NL])
        nc.sync.dma_start(bass.AP(scr["sdram"], 0, [[1, NL]]), kept3[0:1, 0:NL])

        nc.vector.memset(dvec[:], 0.0)
        nc.sync.dma_start(dvec[:, 0:12],
                          bass.AP(scr["sdram"], 0, [[1, 128], [128, 12]]))
        nc.sync.dma_start(dvec[0:24, 12:13],
                          bass.AP(scr["sdram"], 1536, [[1, 24]]))
        nc.vector.memset(drep[:], 0.0)
        nc.gpsimd.partition_broadcast(drep[:, 0:NL], kept3[0:1, 0:NL])
        nc.gpsimd.partition_broadcast(wrep_r[:], kept3[0:1, NL:2*NL])
        nc.gpsimd.partition_broadcast(wrep_i[:], kept3[0:1, 2*NL:3*NL])

    # ---------------- P5: Ht build + v = Ht d ----------------
    vsum = late.tile([128, 2 * NB], F32)
    lam = late.tile([128, 1], F32)
    with tc.tile_pool(name="p5_work", bufs=2) as work:
        nc.vector.memset(vsum[:], 0.0)
        for i in range(NB):
            Gq = work.tile([128, 80], F32, tag="h_gq")
            Iq = work.tile([128, 80], F32, tag="h_iq")
            f_ap = fsc_s[:, i:i+1]
            nc.vector.tensor_scalar(out=Gq[:, 0:40], in0=BF[i][:, 168:208],
                                    scalar1=f_ap, scalar2=None, op0=ALU.mult)
            nc.vector.tensor_scalar(out=Gq[:, 0:40], in0=Gq[:, 0:40],
                                    scalar1=-1.0, scalar2=None, op0=ALU.mult)
            nc.vector.tensor_scalar(out=Gq[:, 40:80], in0=BF[i][:, 40:80],
                                    scalar1=f_ap, scalar2=None, op0=ALU.mult)
            nc.vector.tensor_scalar(out=Iq[:, 0:40], in0=BF[i][:, 128:168],
                                    scalar1=f_ap, scalar2=None, op0=ALU.mult)
            nc.vector.tensor_scalar(out=Iq[:, 0:40], in0=Iq[:, 0:40],
                                    scalar1=-1.0, scalar2=None, op0=ALU.mult)
            nc.vector.tensor_scalar(out=Iq[:, 40:80], in0=BF[i][:, 0:40],
                                    scalar1=f_ap, scalar2=None, op0=ALU.mult)
            Gg_r = work.tile([128, NL], F32, tag="h_ggr")
            Gg_i = work.tile([128, NL], F32, tag="h_ggi")
            qr = work.tile([128, NL], F32, tag="h_qr")
            qi = work.tile([128, NL], F32, tag="h_qi")
            base = 0
            for (t, rs_list) in link_groups:
                o = base
                for (s0, ln) in _contig_segments(rs_list):
                    nc.vector.tensor_copy(Gg_r[:, o:o+ln], Gq[:, s0:s0+ln])
                    nc.vector.tensor_copy(Gg_i[:, o:o+ln], Gq[:, 40+s0:40+s0+ln])
                    o += ln
                base += len(rs_list)
            uniform = (len(link_groups) == 40
                       and all(len(rs) == 39 for _, rs in link_groups))
            if uniform:
                # full-width inc multiply via 0-stride replicated APs
                IncR = Iq[:, 0:40].rearrange("p (t o) -> p t o", o=1
                                             ).broadcast_to([128, 40, 39])
                IncI = Iq[:, 40:80].rearrange("p (t o) -> p t o", o=1
                                              ).broadcast_to([128, 40, 39])
                Gg_r3 = Gg_r[:].rearrange("p (t j) -> p t j", t=40)
                Gg_i3 = Gg_i[:].rearrange("p (t j) -> p t j", t=40)
                qr3 = qr[:].rearrange("p (t j) -> p t j", t=40)
                qi3 = qi[:].rearrange("p (t j) -> p t j", t=40)
                nc.vector.tensor_tensor(out=qr3, in0=Gg_r3, in1=IncR,
                                        op=ALU.mult)
                nc.vector.tensor_tensor(out=qi3, in0=Gg_i3, in1=IncR,
                                        op=ALU.mult)
                nc.vector.tensor_tensor(out=Gg_i3, in0=Gg_i3, in1=IncI,
                                        op=ALU.mult)
                nc.vector.tensor_tensor(out=Gg_r3, in0=Gg_r3, in1=IncI,
                                        op=ALU.mult)
            else:
                base = 0
                for (t, rs_list) in link_groups:
                    sl = slice(base, base + len(rs_list))
                    nc.vector.tensor_scalar(out=qr[:, sl], in0=Gg_r[:, sl],
                                            scalar1=Iq[:, t:t+1], scalar2=None,
                                            op0=ALU.mult)
                    nc.vector.tensor_scalar(out=qi[:, sl], in0=Gg_i[:, sl],
                                            scalar1=Iq[:, t:t+1], scalar2=None,
                                            op0=ALU.mult)
                    nc.vector.tensor_scalar(out=Gg_i[:, sl], in0=Gg_i[:, sl],
                                            scalar1=Iq[:, 40+t:40+t+1],
                                            scalar2=None, op0=ALU.mult)
                    nc.vector.tensor_scalar(out=Gg_r[:, sl], in0=Gg_r[:, sl],
                                            scalar1=Iq[:, 40+t:40+t+1],
                                            scalar2=None, op0=ALU.mult)
                    base += len(rs_list)
            nc.vector.tensor_tensor(out=qr[:], in0=qr[:], in1=Gg_i[:],
                                    op=ALU.subtract)
            nc.vector.tensor_tensor(out=qi[:], in0=qi[:], in1=Gg_r[:],
                                    op=ALU.add)
            hr = work.tile([128, LPAD], F32, tag="h_hr")
            hi = work.tile([128, LPAD], F32, tag="h_hi")
            t2 = work.tile([128, NL], F32, tag="h_t2")
            nc.vector.memset(hr[:], 0.0)
            nc.vector.memset(hi[:], 0.0)
            nc.vector.tensor_tensor(out=hr[:, 0:NL], in0=qr[:], in1=wrep_r[:],
                                    op=ALU.mult)
            nc.vector.tensor_tensor(out=t2[:], in0=qi[:], in1=wrep_i[:],
                                    op=ALU.mult)
            nc.vector.tensor_tensor(out=hr[:, 0:NL], in0=hr[:, 0:NL], in1=t2[:],
                                    op=ALU.subtract)
            nc.vector.tensor_tensor(out=hi[:, 0:NL], in0=qr[:], in1=wrep_i[:],
                                    op=ALU.mult)
            nc.vector.tensor_tensor(out=t2[:], in0=qi[:], in1=wrep_r[:],
                                    op=ALU.mult)
            nc.vector.tensor_tensor(out=hi[:, 0:NL], in0=hi[:, 0:NL], in1=t2[:],
                                    op=ALU.add)
            nc.vector.tensor_scalar(out=hi[:], in0=hi[:], scalar1=-1.0,
                                    scalar2=None, op0=ALU.mult)
            nc.sync.dma_start(scr["htdram"][128*i:128*(i+1), :], hr[:])
            nc.sync.dma_start(scr["htdram"][N+128*i:N+128*(i+1), :], hi[:])
            nc.vector.tensor_tensor(out=t2[:], in0=hr[:, 0:NL],
                                    in1=drep[:, 0:NL], op=ALU.mult)
            nc.vector.tensor_reduce(vsum[:, i:i+1], t2[:], axis=AXX, op=ALU.add)
            nc.vector.tensor_tensor(out=t2[:], in0=hi[:, 0:NL],
                                    in1=drep[:, 0:NL], op=ALU.mult)
            nc.vector.tensor_reduce(vsum[:, NB+i:NB+i+1], t2[:], axis=AXX,
                                    op=ALU.add)
        vsq = work.tile([128, 2 * NB], F32, tag="vsq")
        nc.vector.tensor_tensor(out=vsq[:], in0=vsum[:], in1=vsum[:],
                                op=ALU.mult)
        vred = work.tile([128, 1], F32, tag="vred")
        nc.vector.tensor_reduce(vred[:], vsq[:], axis=AXX, op=ALU.add)
        nc.gpsimd.partition_all_reduce(vred[:], vred[:], 128,
                                       bass_isa.ReduceOp.add)
        nc.scalar.activation(lam[:], vred[:], AF.Sqrt)
        nc.vector.tensor_scalar(out=lam[:], in0=lam[:], scalar1=float(alpha),
                                scalar2=None, op0=ALU.mult)

    # ---------------- P7: Gram ----------------
    st_ = late.tile([128, LB], F32)
    srep = late.tile([128, LPAD], F32)
    with (
        tc.tile_pool(name="g_acc", bufs=1) as gacc,
        tc.tile_pool(name="g_work", bufs=2) as work,
        tc.tile_pool(name="g_psum", bufs=4, space="PSUM") as pg,
    ):
        GA = [gacc.tile([128, LPAD], F32, tag=f"ga{l}", name=f"ga{l}") for l in range(LB)]
        for l in range(LB):
            nc.vector.memset(GA[l][:], 0.0)
        for ch in range(2 * NB):
            htc = work.tile([128, LPAD], F32, tag="g_htc")
            nc.sync.dma_start(htc[:], scr["htdram"][128*ch:128*(ch+1), :])
            htr = work.tile([128, LPAD], F32R, tag="g_htr")
            nc.vector.tensor_copy(htr[:], htc[:])
            for l in range(LB):
                c0 = 128 * l
                for cc in range(c0, LPAD, 416):
                    cw = min(416, LPAD - cc)
                    pgt = pg.tile([128, 416], F32, tag="g_pg")
                    nc.tensor.matmul(pgt[:, 0:cw], htr[:, c0:c0+128],
                                     htr[:, cc:cc+cw], start=True, stop=True)
                    nc.vector.tensor_tensor(out=GA[l][:, cc:cc+cw],
                                            in0=GA[l][:, cc:cc+cw],
                                            in1=pgt[:, 0:cw], op=ALU.add)
        for l in range(LB):
            nc.sync.dma_start(scr["gramdram"][128*l:128*(l+1), :], GA[l][:])
        gd = work.tile([128, LB], F32, tag="gd")
        nc.sync.dma_start(gd[:], bass.AP(scr["gramdram"], 0,
                                         [[LPAD + 1, 128],
                                          [128 * (LPAD + 1), LB]]))
        nc.vector.tensor_scalar(out=gd[:], in0=gd[:], scalar1=lam[:],
                                scalar2=None, op0=ALU.add)
        nc.scalar.activation(st_[:], gd[:], AF.Sqrt)
        nc.vector.reciprocal(st_[:], st_[:])
        ps_ = pg.tile([LB, 128], F32, tag="s_ps")
        nc.tensor.matmul(ps_[:], st_[:], id_s[:], start=True, stop=True)
        s13 = work.tile([LB, 128], F32, tag="s13")
        nc.vector.tensor_copy(s13[:], ps_[:])
        nc.sync.dma_start(bass.AP(scr["srowdram"], 0, [[1, LPAD]]), s13[:])
        srow = work.tile([1, LPAD], F32, tag="srow")
        nc.sync.dma_start(srow[:], bass.AP(scr["srowdram"], 0, [[1, LPAD]]))
        nc.gpsimd.partition_broadcast(srep[:], srow[:])

    # ---------------- P8: scaled SPD solve ----------------
    bf2_pool = ctx.enter_context(tc.tile_pool(name="bf2", bufs=1))
    BF2 = [bf2_pool.tile([128, 128], F32R, tag=f"bf2_{l}", name=f"bf2_{l}") for l in range(LB)]
    with (
        tc.tile_pool(name="s_tri", bufs=1) as tri2,
        tc.tile_pool(name="s_work", bufs=2) as work,
        tc.tile_pool(name="s_pmm", bufs=2, space="PSUM") as pmm,
        tc.tile_pool(name="s_pmisc", bufs=1, space="PSUM") as pmisc,
    ):
        dsc = work.tile([128, LB], F32, tag="dsc")
        nc.vector.tensor_tensor(out=dsc[:], in0=dvec[:], in1=st_[:], op=ALU.mult)
        zz = work.tile([128, 128], F32, tag="zz")
        nc.vector.memset(zz[:], 0.0)
        for l in range(LB):
            nc.vector.tensor_copy(BF2[l][:], zz[:])
            nc.vector.tensor_copy(BF2[l][:, 0:1], dsc[:, l:l+1])
        GT = {}
        for i in range(LB):
            for j in range(i, LB):
                GT[(i, j)] = tri2.tile([128, 128], F32R, tag=f"g{i}_{j}", name=f"g{i}_{j}")
                gload = work.tile([128, 128], F32, tag="g_load")
                nc.sync.dma_start(gload[:],
                                  scr["gramdram"][128*i:128*(i+1),
                                                  128*j:128*(j+1)])
                nc.vector.tensor_scalar(out=gload[:], in0=gload[:],
                                        scalar1=st_[:, i:i+1], scalar2=None,
                                        op0=ALU.mult)
                nc.vector.tensor_tensor(out=gload[:], in0=gload[:],
                                        in1=srep[:, 128*j:128*(j+1)],
                                        op=ALU.mult)
                if i == j:
                    ones1 = work.tile([128, 1], F32, tag="diag1")
                    nc.vector.memset(ones1[:], 1.0)
                    nc.vector.copy_predicated(gload[:], idu_s[:],
                                              ones1[:].broadcast_to([128, 128]))
                nc.vector.tensor_copy(GT[(i, j)][:], gload[:])
        for k in range(LB):
            V = work.tile([128, 128], F32R, tag="lu2_V")
            _newton_real(nc, work, pmm, pmisc, GT[(k, k)], V, id_s, NEWTON_SPD)
            nc.sync.dma_start(scr["v2dram"][128*k:128*(k+1), :], V[:])
            for i in range(k + 1, LB):
                ptr = pmisc.tile([128, 128], F32R, tag="lu2_ptr")
                nc.tensor.transpose(ptr[:], GT[(k, i)][:], idr_s[:])
                utt = work.tile([128, 128], F32R, tag="lu2_utt")
                nc.vector.tensor_copy(utt[:], ptr[:])
                nc.sync.dma_start(
                    scr["ut2dram"][128*i:128*(i+1), 128*k:128*(k+1)], utt[:])
            for i in range(k + 1, LB):
                pl = pmm.tile([128, 128], F32, tag="cmmp1")
                nc.tensor.matmul(pl[:], V[:], GT[(k, i)][:], start=True,
                                 stop=True)
                LT = work.tile([128, 128], F32R, tag="lu2_LT")
                nc.vector.tensor_copy(LT[:], pl[:])
                pb = pmm.tile([128, 128], F32, tag="cmmp2")
                nc.tensor.matmul(pb[:], LT[:], BF2[k][:], start=True, stop=True)
                nc.vector.tensor_tensor(out=BF2[i][:], in0=BF2[i][:],
                                        in1=pb[:], op=ALU.subtract)
                for j in range(i, LB):
                    pt_ = pmm.tile([128, 128], F32, tag="cmmp1")
                    nc.tensor.matmul(pt_[:], LT[:], GT[(k, j)][:], start=True,
                                     stop=True)
                    nc.vector.tensor_tensor(out=GT[(i, j)][:],
                                            in0=GT[(i, j)][:], in1=pt_[:],
                                            op=ALU.subtract)

    ys = late.tile([128, LB], F32)
    yrep = late.tile([128, LPAD], F32)
    with (
        tc.tile_pool(name="b2_work", bufs=3) as work,
        tc.tile_pool(name="b2_pacc", bufs=1, space="PSUM") as pacc,
        tc.tile_pool(name="b2_pmm", bufs=2, space="PSUM") as pmm,
    ):
        for k in range(LB - 1, -1, -1):
            W = work.tile([128, 128], F32R, tag="bs2_W")
            nc.vector.tensor_copy(W[:], BF2[k][:])
            if k < LB - 1:
                P1 = pacc.tile([128, 128], F32, tag="bs2_p1")
                for idx, j in enumerate(range(k + 1, LB)):
                    utt = work.tile([128, 128], F32R, tag="bs2_utt")
                    nc.sync.dma_start(
                        utt[:], scr["ut2dram"][128*j:128*(j+1),
                                               128*k:128*(k+1)])
                    nc.tensor.matmul(P1[:], utt[:], BF2[j][:],
                                     start=(idx == 0), stop=(j == LB - 1))
                nc.vector.tensor_tensor(out=W[:], in0=W[:], in1=P1[:],
                                        op=ALU.subtract)
            Vk = work.tile([128, 128], F32R, tag="bs2_V")
            nc.sync.dma_start(Vk[:], scr["v2dram"][128*k:128*(k+1), :])
            Pf = pmm.tile([128, 128], F32, tag="bs2_pf")
            nc.tensor.matmul(Pf[:], Vk[:], W[:], start=True, stop=True)
            nc.vector.tensor_copy(BF2[k][:], Pf[:])
        for l in range(LB):
            nc.vector.tensor_copy(ys[:, l:l+1], BF2[l][:, 0:1])
        nc.vector.tensor_tensor(out=ys[:], in0=ys[:], in1=st_[:], op=ALU.mult)
        psy = pmm.tile([LB, 128], F32, tag="y_ps")
        nc.tensor.matmul(psy[:], ys[:], id_s[:], start=True, stop=True)
        y13 = work.tile([LB, 128], F32, tag="y13")
        nc.vector.tensor_copy(y13[:], psy[:])
        nc.sync.dma_start(bass.AP(scr["yrowdram"], 0, [[1, LPAD]]), y13[:])
        yrow = work.tile([1, LPAD], F32, tag="yrow")
        nc.sync.dma_start(yrow[:], bass.AP(scr["yrowdram"], 0, [[1, LPAD]]))
        nc.gpsimd.partition_broadcast(yrep[:], yrow[:])

    # ---------------- P9: chi = Ht y ----------------
    with tc.tile_pool(name="p9_work", bufs=2) as work:
        chi = late.tile([128, 2 * NB], F32)
        for ch in range(2 * NB):
            htc = work.tile([128, LPAD], F32, tag="c_htc")
            nc.sync.dma_start(htc[:], scr["htdram"][128*ch:128*(ch+1), :])
            tm = work.tile([128, LPAD], F32, tag="c_tm")
            nc.vector.tensor_tensor(out=tm[:], in0=htc[:], in1=yrep[:],
                                    op=ALU.mult)
            nc.vector.tensor_reduce(chi[:, ch:ch+1], tm[:], axis=AXX,
                                    op=ALU.add)
        nc.sync.dma_start(bass.AP(out_chi, 0, [[1, 128], [128, 2 * NB]]),
                          chi[:])
    ctx.close()


_GBASE = {}

def _contig_segments(rs_list):
    segs = []
    s = rs_list[0]; prev = s
    for r in rs_list[1:]:
        if r == prev + 1:
            prev = r
        else:
            segs.append((s, prev - s + 1)); s = r; prev = r
    segs.append((s, prev - s + 1))
    return segs


_CACHED = {}


def kernel(epsilon_r_iter, chi_iter, total_power, alpha, grid_x, grid_y,
           direct_field, incident_field, G_freespace, G_freespace_scaled,
           sensor_links):
    eps = np.asarray(epsilon_r_iter)
    chi_it = np.asarray(chi_iter)
    tp = np.asarray(total_power, dtype=np.float32)
    alpha_f = float(np.asarray(alpha))
    gx = np.asarray(grid_x, dtype=np.float32)
    gy = np.asarray(grid_y, dtype=np.float32)
    df = np.asarray(direct_field)
    einc = np.asarray(incident_field)
    gfs = np.asarray(G_freespace)
    gsc = np.asarray(G_freespace_scaled)
    links = np.asarray(sensor_links)

    x = gx.T.reshape(N).astype(np.float32)
    y = gy.T.reshape(N).astype(np.float32)
    scat = np.real(eps.T.reshape(N)).astype(np.float32)

    geomS = np.stack([np.ones(N, np.float32), -2.0*x, -2.0*y,
                      (x*x + y*y)]).astype(np.float32)
    geomR = np.stack([(x*x + y*y), x, y,
                      np.ones(N, np.float32)]).astype(np.float32)
    scat_t = scat.reshape(NB, 128).T.copy()

    bpack = np.zeros((N, RW), np.float32)
    bpack[:, 0:40] = -einc.real; bpack[:, 40:80] = -gfs.real
    bpack[:, 128:168] = -einc.imag; bpack[:, 168:208] = -gfs.imag
    gscT = np.concatenate([gsc.real.T, gsc.imag.T], axis=1).astype(np.float32)
    dfpack = np.concatenate([df.real, df.imag], axis=1).astype(np.float32)
    tpT = tp.T.copy().astype(np.float32)

    groups = []
    i = 0
    while i < len(links):
        t = int(links[i, 0])
        rs_list = []
        while i < len(links) and int(links[i, 0]) == t:
            rs_list.append(int(links[i, 1]))
            i += 1
        groups.append((t, rs_list))

    _GBASE.clear()
    o = 0
    for (t, rs_list) in groups:
        _GBASE[t] = o
        o += len(rs_list)
    key = (hash(links.tobytes()), alpha_f)
    if key not in _CACHED:
        _CACHED[key] = build_program(groups, alpha_f)
    nc = _CACHED[key]

    id128 = np.eye(128, dtype=np.float32)
    im = {
        "geomS": geomS, "geomR": geomR, "scat_t": scat_t, "bpack": bpack,
        "gscT": gscT, "dfpack": dfpack, "tpT": tpT,
        "id128": id128, "idu8": id128.astype(np.uint8),
    }
    import os as _os
    _tr = _os.environ.get("KTRACE", "0") == "1"
    res = run_bass_kernel_spmd(nc, [im] * 8, core_ids=list(range(8)), trace=_tr)
    out = res.results[0]
    _CACHED["last"] = (res, out)

    chi = out["out_chi"]
    dchi_r = chi[:N].reshape(M, M).T
    dchi_i = chi[N:].reshape(M, M).T
    chi_new = (chi_it + (dchi_r + 1j * dchi_i)).astype(np.complex64)
    return chi_new + 1.0, chi_new



# revision 9
# speedup vs baseline: 2.1816x; 2.1816x over previous
"""DRIM layer (distorted Rytov inverse-scattering iteration) on Trainium2.

Optimized single-core program replicated SPMD on 8 cores.  Key design:
  - all bulk matrix state (Z, factors, rhs, H) stored bf16 in SBUF/DRAM;
    fp32 PSUM accumulation everywhere (validated end-to-end ~1e-3)
  - elementwise work split across DVE (vector) and Pool (gpsimd) engines
  - complex products via plane-swapped (-im|re) rhs copies so each complex
    matmul is 2 wide PSUM-accumulating matmuls, one combine op
  - sin/cos range reduction via one fused (x+pi mod 2pi) tensor_scalar
  - activation-table churn avoided (two-pass Z build: sqrt pass, sin pass)
  - Newton block inversions emitted interleaved with trailing updates
  - pivot-row transposes via XBAR DMA-transpose loads (no PE transposes)
  - Gram accumulated over 4-row-chunk quads in PSUM
"""
import math
import numpy as np

import concourse.bass as bass
import concourse.bacc as bacc
import concourse.bass_isa as bass_isa
import concourse.mybir as mybir
import concourse.tile as tile
from concourse.bass_utils import run_bass_kernel_spmd

F32 = mybir.dt.float32
F32R = mybir.dt.float32r
BF16 = mybir.dt.bfloat16
U8 = mybir.dt.uint8
I32 = mybir.dt.int32
AF = mybir.ActivationFunctionType
ALU = mybir.AluOpType
AXX = mybir.AxisListType.X

M = 48
N = M * M
NB = N // 128               # 18
TX = RX = 40
L16 = 1600                  # 40x40 links incl. zero-weighted diagonal
LPAD = 1664
LB = LPAD // 128            # 13
RW = 256
DOI = 3.0
WL = 0.125
K0 = 2.0 * math.pi / WL
IMP = 120.0 * math.pi
GRID_LEN = DOI / M
GRID_RADIUS = math.sqrt(GRID_LEN ** 2 / math.pi)
NOISE = 1e-6

def _j1s(x):
    t2 = (x / 3.0) ** 2
    return x * (0.5 - 0.56249985*t2 + 0.21093573*t2**2 - 0.03954289*t2**3
                + 0.00443319*t2**4 - 0.00031761*t2**5 + 0.00001109*t2**6)

def _y1s(x):
    t2 = (x / 3.0) ** 2
    p = (-0.6366198 + 0.2212091*t2 + 2.1682709*t2**2 - 1.3164827*t2**3
         + 0.3123951*t2**4 - 0.0400976*t2**5 + 0.0027873*t2**6)
    return ((2.0/math.pi) * x * math.log(0.5*x) * _j1s(x) + p) / x

X0C = K0 * GRID_RADIUS
GRID_AREA = 4.0*math.pi*GRID_RADIUS/(2.0*K0) * _j1s(X0C)
C1 = -IMP * math.pi * GRID_RADIUS / 2.0
C2 = _j1s(X0C)
C3R, C3I = _j1s(X0C), _y1s(X0C)
C1C2 = C1 * C2
ZD_RE = C1 * C3R
ZD_IM_C = C1 * C3I
SA = GRID_AREA * K0 * K0
TWO_PI = 2.0 * math.pi
INV_2PI = 1.0 / TWO_PI
LOG10E20 = 20.0 * math.log10(math.e)
CADD = 10.0 * math.log10(WL * WL / (4.0 * math.pi * IMP) / 1e-3)
C20L = 20.0 / math.log(10.0)

F0C = [0.79788456, -0.00000077, -0.00552740, -0.00009512]
THC = [-0.78539816, -0.04166397, -0.00003954, 0.00262573]
F0CS = [c * (3.0 ** k) * C1C2 for k, c in enumerate(F0C)]
THCS = [c * (3.0 ** k) for k, c in enumerate(THC)]

NEWTON_Z = 17
NEWTON_SPD = 14


class Mux:
    """Alternate elementwise ops between DVE (vector) and Pool (gpsimd)."""
    def __init__(self, nc):
        self.nc = nc
        self.i = 0

    def eng(self):
        self.i += 1
        return self.nc.vector if (self.i & 1) else self.nc.gpsimd


def build_program(alpha):
    nc = bacc.Bacc("TRN2", target_bir_lowering=False, num_devices=8)
    din = {}
    def inp(name, shape, dtype=F32):
        din[name] = nc.dram_tensor(name, shape, dtype, kind="ExternalInput")
    inp("geomS", [4, N]); inp("geomR", [4, N]); inp("scat_t", [128, NB])
    inp("bpack", [N, RW]); inp("gscT", [N, 80]); inp("dfpack", [40, 80])
    inp("tp40", [40, 40]); inp("id128", [128, 128]); inp("idu8", [128, 128], U8)
    out_chi = nc.dram_tensor("out_chi", [2 * N], F32, kind="ExternalOutput")
    xdbg = nc.dram_tensor("xdbg", [N, RW], BF16, kind="ExternalOutput")
    tfdbg = nc.dram_tensor("tfdbg", [40, 80], F32, kind="ExternalOutput")
    scr = {}
    scr["utdram"] = nc.dram_tensor("utdram", [N, 2 * N], BF16, kind="Internal")
    scr["htdram"] = nc.dram_tensor("htdram", [2 * N, LPAD], BF16, kind="Internal")
    scr["sdram"] = nc.dram_tensor("sdram", [L16], F32, kind="Internal")
    scr["wrdram"] = nc.dram_tensor("wrdram", [L16], F32, kind="Internal")
    scr["widram"] = nc.dram_tensor("widram", [L16], F32, kind="Internal")
    scr["srowdram"] = nc.dram_tensor("srowdram", [LPAD], F32, kind="Internal")
    scr["yrowdram"] = nc.dram_tensor("yrowdram", [LPAD], F32, kind="Internal")

    with tile.TileContext(nc) as tc:
        _body(nc, tc, din, out_chi, xdbg, tfdbg, scr, alpha)
    nc.compile()
    return nc


def _newton_scale(nc, work, pmisc, m, tag):
    """a = 1/(max rowsum)^2 of |m| (m symmetric) -> [128,1] f32 AP."""
    cs = work.tile([128, 1], F32, tag=f"nwcs_{tag}")
    nc.vector.tensor_reduce(cs[:], m[:], axis=AXX, op=ALU.add)
    nc.gpsimd.partition_all_reduce(cs[:], cs[:], 128, bass_isa.ReduceOp.max)
    a = work.tile([128, 1], F32, tag=f"nwa_{tag}")
    nc.vector.tensor_tensor(out=a[:], in0=cs[:], in1=cs[:], op=ALU.mult)
    nc.vector.reciprocal(a[:], a[:])
    return a


def _newton_cplx_steps(nc, work, pmm, pmisc, Dap, consts, iters):
    """Generator of emission closures for one complex Newton inversion.

    Dap: [128,256] bf16 (re|im) block, symmetric; V is written back to Dap.
    """
    st = {}

    def prologue():
        m1 = work.tile([128, 128], F32, tag="nw_m1")
        m2 = work.tile([128, 128], F32, tag="nw_m2")
        nc.scalar.activation(m1[:], Dap[:, 0:128], AF.Abs)
        nc.scalar.activation(m2[:], Dap[:, 128:256], AF.Abs)
        nc.vector.tensor_tensor(out=m1[:], in0=m1[:], in1=m2[:], op=ALU.max)
        a = _newton_scale(nc, work, pmisc, m1, "c")
        X = work.tile([128, RW], BF16, tag="nw_X")
        XB = work.tile([128, RW], BF16, tag="nw_XB")
        DN = work.tile([128, 128], BF16, tag="nw_DN")
        nc.vector.tensor_scalar(out=X[:, 0:128], in0=Dap[:, 0:128],
                                scalar1=a[:], scalar2=None, op0=ALU.mult)
        nc.vector.tensor_scalar(out=X[:, 128:256], in0=Dap[:, 128:256],
                                scalar1=a[:], scalar2=-1.0, op0=ALU.mult,
                                op1=ALU.mult)
        nc.gpsimd.tensor_scalar(out=XB[:, 0:128], in0=Dap[:, 128:256],
                                scalar1=a[:], scalar2=None, op0=ALU.mult)
        nc.gpsimd.tensor_scalar(out=XB[:, 128:256], in0=Dap[:, 0:128],
                                scalar1=a[:], scalar2=None, op0=ALU.mult)
        nc.gpsimd.tensor_scalar(out=DN[:], in0=Dap[:, 128:256], scalar1=-1.0,
                                scalar2=None, op0=ALU.mult)
        st["X"], st["XB"], st["DN"] = X, XB, DN

    yield prologue

    def one_iter():
        X, XB, DN = st["X"], st["XB"], st["DN"]
        PP = pmm.tile([128, RW], F32, tag="nw_PP")
        nc.tensor.matmul(PP[:], Dap[:, 0:128], X[:], start=True, stop=False)
        nc.tensor.matmul(PP[:], Dap[:, 128:256], XB[:], start=False, stop=True)
        PB = pmm.tile([128, RW], F32, tag="nw_PB")
        nc.tensor.matmul(PB[:], Dap[:, 0:128], XB[:], start=True, stop=False)
        nc.tensor.matmul(PB[:], DN[:], X[:], start=False, stop=True)
        R = work.tile([128, RW], BF16, tag="nw_R")
        RB = work.tile([128, RW], BF16, tag="nw_RB")
        nc.vector.tensor_tensor(out=R[:], in0=consts["Ip"][:], in1=PP[:],
                                op=ALU.subtract)
        nc.gpsimd.tensor_tensor(out=RB[:], in0=consts["Iq"][:], in1=PB[:],
                                op=ALU.subtract)
        QQ = pmm.tile([128, RW], F32, tag="nw_QQ")
        nc.tensor.matmul(QQ[:], X[:, 0:128], R[:], start=True, stop=False)
        nc.tensor.matmul(QQ[:], X[:, 128:256], RB[:], start=False, stop=True)
        nc.vector.tensor_tensor(out=X[:], in0=X[:], in1=QQ[:], op=ALU.add)
        nc.gpsimd.tensor_scalar(out=XB[:, 0:128], in0=X[:, 128:256],
                                scalar1=-1.0, scalar2=None, op0=ALU.mult)
        nc.gpsimd.tensor_copy(XB[:, 128:256], X[:, 0:128])

    for _ in range(iters):
        yield one_iter

    def final():
        nc.vector.tensor_copy(Dap[:], st["X"][:])

    yield final


def _newton_real_steps(nc, work, pmm, pmisc, Dap, consts, iters):
    """Same for a real symmetric [128,128] bf16 block; V written to Dap."""
    st = {}

    def prologue():
        m1 = work.tile([128, 128], F32, tag="nw2_m1")
        nc.scalar.activation(m1[:], Dap[:], AF.Abs)
        a = _newton_scale(nc, work, pmisc, m1, "r")
        X = work.tile([128, 128], BF16, tag="nw2_X")
        nc.vector.tensor_scalar(out=X[:], in0=Dap[:], scalar1=a[:],
                                scalar2=None, op0=ALU.mult)
        st["X"] = X

    yield prologue

    def one_iter():
        X = st["X"]
        PP = pmm.tile([128, 128], F32, tag="nw2_PP")
        nc.tensor.matmul(PP[:], Dap[:], X[:], start=True, stop=True)
        R = work.tile([128, 128], BF16, tag="nw2_R")
        nc.vector.tensor_tensor(out=R[:], in0=consts["Ib"][:], in1=PP[:],
                                op=ALU.subtract)
        QQ = pmm.tile([128, 128], F32, tag="nw2_QQ")
        nc.tensor.matmul(QQ[:], X[:], R[:], start=True, stop=True)
        nc.gpsimd.tensor_tensor(out=X[:], in0=X[:], in1=QQ[:], op=ALU.add)

    for _ in range(iters):
        yield one_iter

    def final():
        nc.vector.tensor_copy(Dap[:], st["X"][:])

    yield final


def _body(nc, tc, din, out_chi, xdbg, tfdbg, scr, alpha):
    import contextlib
    ctx = contextlib.ExitStack()
    mux = Mux(nc)

    consts_pool = ctx.enter_context(tc.tile_pool(name="consts", bufs=1))
    id_s = consts_pool.tile([128, 128], F32)
    nc.sync.dma_start(id_s[:], din["id128"][:])
    idu_s = consts_pool.tile([128, 128], U8)
    nc.sync.dma_start(idu_s[:], din["idu8"][:])
    idb_s = consts_pool.tile([128, 128], BF16)      # +I bf16
    nc.vector.tensor_copy(idb_s[:], id_s[:])
    nidb_s = consts_pool.tile([128, 128], BF16)     # -I bf16
    nc.gpsimd.tensor_scalar(out=nidb_s[:], in0=id_s[:], scalar1=-1.0,
                            scalar2=None, op0=ALU.mult)
    Ip_s = consts_pool.tile([128, RW], BF16)        # (I|0)
    nc.vector.memset(Ip_s[:], 0.0)
    nc.vector.tensor_copy(Ip_s[:, 0:128], id_s[:])
    Iq_s = consts_pool.tile([128, RW], BF16)        # (0|I)
    nc.gpsimd.memset(Iq_s[:], 0.0)
    nc.gpsimd.tensor_copy(Iq_s[:, 128:256], id_s[:])
    scat_s = consts_pool.tile([128, NB], F32)
    nc.sync.dma_start(scat_s[:], din["scat_t"][:])

    zdi_s = consts_pool.tile([128, NB], BF16)
    fsc_s = consts_pool.tile([128, NB], F32)
    t0 = consts_pool.tile([128, NB], F32)
    nc.vector.tensor_scalar(out=t0[:], in0=scat_s[:], scalar1=-1.0,
                            scalar2=None, op0=ALU.add)
    nc.vector.reciprocal(t0[:], t0[:])
    nc.vector.tensor_scalar(out=fsc_s[:], in0=t0[:], scalar1=(IMP / K0),
                            scalar2=None, op0=ALU.mult)
    nc.vector.tensor_tensor(out=t0[:], in0=t0[:], in1=scat_s[:], op=ALU.mult)
    nc.vector.tensor_scalar(out=zdi_s[:], in0=t0[:], scalar1=-(IMP / K0),
                            scalar2=ZD_IM_C, op0=ALU.mult, op1=ALU.add)
    zdr_c = consts_pool.tile([128, 1], BF16)
    nc.vector.memset(zdr_c[:], float(ZD_RE))
    npi_c = consts_pool.tile([128, 1], F32)
    nc.vector.memset(npi_c[:], -math.pi)

    bf_pool = ctx.enter_context(tc.tile_pool(name="bf", bufs=1))
    BF = [bf_pool.tile([128, RW], BF16, tag=f"bf{i}", name=f"bf{i}")
          for i in range(NB)]

    consts = {"Ip": Ip_s, "Iq": Iq_s, "Ib": idb_s, "nIb": nidb_s}

    with tc.tile_pool(name="tri", bufs=1) as tri:
        ZR = [tri.tile([128, (NB - i) * RW], BF16, tag=f"zr{i}", name=f"zr{i}")
              for i in range(NB)]

        # ---------------- P1: Z build ----------------
        with (
            tc.tile_pool(name="zb_geom", bufs=2) as gpool,
            tc.tile_pool(name="zb_scr", bufs=1) as spool,
            tc.tile_pool(name="zb_work", bufs=2) as work,
            tc.tile_pool(name="zb_psum", bufs=3, space="PSUM") as pz,
        ):
            th_s = spool.tile([128, N], F32, name="th_s")
            amp_s = spool.tile([128, N], BF16, name="amp_s")
            for k in range(NB):
                r0 = 128 * k
                Wr = (NB - k) * 128
                gS = gpool.tile([4, 128], F32, tag="gS", name="gS")
                nc.sync.dma_start(gS[:], din["geomS"][:, r0:r0+128])
                # pass A: distances, polynomials, amplitude, phase
                for c in range(0, Wr, 512):
                    w = min(512, Wr - c)
                    gR = work.tile([4, 512], F32, tag="gR")
                    nc.sync.dma_start(gR[:, 0:w], din["geomR"][:, r0+c:r0+c+w])
                    pd = pz.tile([128, 512], F32, tag="zb_pd")
                    nc.tensor.matmul(pd[:, 0:w], gS[:].bitcast(F32R),
                                     gR[:, 0:w].bitcast(F32R),
                                     start=True, stop=True)
                    dsq = work.tile([128, 512], F32, tag="zb_dsq")
                    nc.gpsimd.tensor_scalar(out=dsq[:, 0:w], in0=pd[:, 0:w],
                                            scalar1=0.002, scalar2=None,
                                            op0=ALU.max)
                    x = work.tile([128, 512], F32, tag="zb_x")
                    nc.scalar.activation(x[:, 0:w], dsq[:, 0:w], AF.Sqrt,
                                         scale=float(K0 * K0))
                    sp = work.tile([128, 512], F32, tag="zb_sp")
                    nc.vector.reciprocal(sp[:, 0:w], x[:, 0:w])
                    s2 = work.tile([128, 512], F32, tag="zb_s2")
                    nc.gpsimd.tensor_tensor(out=s2[:, 0:w], in0=sp[:, 0:w],
                                            in1=sp[:, 0:w], op=ALU.mult)
                    t1 = work.tile([128, 512], F32, tag="zb_t1")
                    nc.vector.tensor_scalar(out=t1[:, 0:w], in0=sp[:, 0:w],
                                            scalar1=THCS[1], scalar2=THCS[0],
                                            op0=ALU.mult, op1=ALU.add)
                    t2 = work.tile([128, 512], F32, tag="zb_t2")
                    nc.gpsimd.tensor_scalar(out=t2[:, 0:w], in0=sp[:, 0:w],
                                            scalar1=THCS[3], scalar2=THCS[2],
                                            op0=ALU.mult, op1=ALU.add)
                    nc.vector.tensor_tensor(out=t1[:, 0:w], in0=t1[:, 0:w],
                                            in1=x[:, 0:w], op=ALU.add)
                    nc.gpsimd.tensor_tensor(out=t2[:, 0:w], in0=t2[:, 0:w],
                                            in1=s2[:, 0:w], op=ALU.mult)
                    nc.vector.tensor_tensor(out=th_s[:, c:c+w], in0=t1[:, 0:w],
                                            in1=t2[:, 0:w], op=ALU.add)
                    u1 = work.tile([128, 512], F32, tag="zb_u1")
                    nc.gpsimd.tensor_scalar(out=u1[:, 0:w], in0=sp[:, 0:w],
                                            scalar1=F0CS[1], scalar2=F0CS[0],
                                            op0=ALU.mult, op1=ALU.add)
                    u2 = work.tile([128, 512], F32, tag="zb_u2")
                    nc.vector.tensor_scalar(out=u2[:, 0:w], in0=sp[:, 0:w],
                                            scalar1=F0CS[3], scalar2=F0CS[2],
                                            op0=ALU.mult, op1=ALU.add)
                    sqx = work.tile([128, 512], F32, tag="zb_sqx")
                    nc.scalar.activation(sqx[:, 0:w], sp[:, 0:w], AF.Sqrt)
                    nc.vector.tensor_tensor(out=u2[:, 0:w], in0=u2[:, 0:w],
                                            in1=s2[:, 0:w], op=ALU.mult)
                    nc.gpsimd.tensor_tensor(out=u1[:, 0:w], in0=u1[:, 0:w],
                                            in1=u2[:, 0:w], op=ALU.add)
                    nc.gpsimd.tensor_tensor(out=amp_s[:, c:c+w],
                                            in0=u1[:, 0:w], in1=sqx[:, 0:w],
                                            op=ALU.mult)
                # pass B: sines into ZR row (strided per-plane writes)
                for c in range(0, Wr, 512):
                    w = min(512, Wr - c)
                    nblk = w // 128
                    sa = work.tile([128, 512], F32, tag="zb_sa")
                    nc.vector.tensor_scalar(out=sa[:, 0:w], in0=th_s[:, c:c+w],
                                            scalar1=math.pi, scalar2=TWO_PI,
                                            op0=ALU.add, op1=ALU.mod)
                    sinr = work.tile([128, 512], F32, tag="zb_sin")
                    nc.scalar.activation(sinr[:, 0:w], sa[:, 0:w], AF.Sin,
                                         bias=npi_c[:])
                    sa2 = work.tile([128, 512], F32, tag="zb_sa2")
                    nc.gpsimd.tensor_scalar(out=sa2[:, 0:w], in0=th_s[:, c:c+w],
                                            scalar1=1.5*math.pi, scalar2=TWO_PI,
                                            op0=ALU.add, op1=ALU.mod)
                    cosr = work.tile([128, 512], F32, tag="zb_cos")
                    nc.scalar.activation(cosr[:, 0:w], sa2[:, 0:w], AF.Sin,
                                         bias=npi_c[:])
                    zr3 = ZR[k][:, 2*c:2*c+nblk*RW].rearrange(
                        "p (n t) -> p n t", t=RW)
                    s3 = sinr[:, 0:w].rearrange("p (n t) -> p n t", t=128)
                    c3 = cosr[:, 0:w].rearrange("p (n t) -> p n t", t=128)
                    a3 = amp_s[:, c:c+w].rearrange("p (n t) -> p n t", t=128)
                    nc.gpsimd.tensor_tensor(out=zr3[:, :, 0:128], in0=c3,
                                            in1=a3, op=ALU.mult)
                    nc.vector.tensor_tensor(out=zr3[:, :, 128:256], in0=s3,
                                            in1=a3, op=ALU.mult)
                # diagonal overrides
                nc.vector.copy_predicated(
                    ZR[k][:, 0:128], idu_s[:],
                    zdr_c[:].broadcast_to([128, 128]))
                nc.vector.copy_predicated(
                    ZR[k][:, 128:256], idu_s[:],
                    zdi_s[:, k:k+1].broadcast_to([128, 128]))

        # ---------------- P2: block LDL^T ----------------
        with (
            tc.tile_pool(name="lu_zb", bufs=1) as zbpool,
            tc.tile_pool(name="lu_work", bufs=2) as work,
            tc.tile_pool(name="lu_nw", bufs=1) as nwork,
            tc.tile_pool(name="lu_pmm", bufs=1, space="PSUM") as pmm,
            tc.tile_pool(name="lu_pup", bufs=3, space="PSUM") as pup,
        ):
            pmisc = None
            ZB = zbpool.tile([128, (NB - 1) * RW], BF16, name="zbswap")
            BFB = zbpool.tile([128, RW], BF16, name="bfbswap")
            ldtmp = work.tile([128, RW], F32, tag="ldtmp")
            for i in range(NB):
                nc.sync.dma_start(ldtmp[:], din["bpack"][128*i:128*(i+1), :])
                nc.vector.tensor_copy(BF[i][:], ldtmp[:])
                ldtmp = work.tile([128, RW], F32, tag="ldtmp")

            pending = []

            def drain(n):
                for _ in range(min(n, len(pending))):
                    pending.pop(0)()

            for step in _newton_cplx_steps(nc, nwork, pmm, pmisc,
                                           ZR[0][:, 0:RW], consts, NEWTON_Z):
                step()

            for k in range(NB):
                nr = NB - 1 - k      # trailing rows
                if nr > 0:
                    # swapped pivot row (-im|re) for blocks k+1..17
                    zb3 = ZB[:, 0:nr*RW].rearrange("p (n t) -> p n t", t=RW)
                    zr3 = ZR[k][:, RW:(nr+1)*RW].rearrange(
                        "p (n t) -> p n t", t=RW)
                    nc.vector.tensor_scalar(out=zb3[:, :, 0:128],
                                            in0=zr3[:, :, 128:256],
                                            scalar1=-1.0, scalar2=None,
                                            op0=ALU.mult)
                    nc.gpsimd.tensor_copy(zb3[:, :, 128:256], zr3[:, :, 0:128])
                    # swapped pivot rhs
                    nc.vector.tensor_scalar(out=BFB[:, 0:128],
                                            in0=BF[k][:, 128:256],
                                            scalar1=-1.0, scalar2=None,
                                            op0=ALU.mult)
                    nc.gpsimd.tensor_copy(BFB[:, 128:256], BF[k][:, 0:128])
                    # store pivot row for backsolve (transposed on load)
                    nc.sync.dma_start(
                        scr["utdram"][128*k:128*(k+1), RW*(k+1):RW*NB],
                        ZR[k][:, RW:(nr+1)*RW])
                for i in range(k + 1, NB):
                    off = (i - k) * RW
                    zoff = (i - k - 1) * RW
                    PL = pmm.tile([128, RW], F32, tag="lu_PL")
                    nc.tensor.matmul(PL[:], ZR[k][:, 0:128],
                                     ZR[k][:, off:off+RW], start=True,
                                     stop=False)
                    nc.tensor.matmul(PL[:], ZR[k][:, 128:256],
                                     ZB[:, zoff:zoff+RW], start=False,
                                     stop=True)
                    LT = work.tile([128, RW], BF16, tag="lu_LT")
                    mux.eng().tensor_copy(LT[:], PL[:])
                    # rhs update
                    PBf = pmm.tile([128, RW], F32, tag="lu_PBf")
                    nc.tensor.matmul(PBf[:], LT[:, 0:128], BF[k][:],
                                     start=True, stop=False)
                    nc.tensor.matmul(PBf[:], LT[:, 128:256], BFB[:],
                                     start=False, stop=True)
                    mux.eng().tensor_tensor(out=BF[i][:], in0=BF[i][:],
                                            in1=PBf[:], op=ALU.subtract)
                    # trailing row update, 512-wide chunks
                    Wi = (NB - i) * RW
                    for c in range(0, Wi, 512):
                        w = min(512, Wi - c)
                        PU = pup.tile([128, 512], F32, tag="lu_PU")
                        nc.tensor.matmul(PU[:, 0:w], LT[:, 0:128],
                                         ZR[k][:, off+c:off+c+w],
                                         start=True, stop=False)
                        nc.tensor.matmul(PU[:, 0:w], LT[:, 128:256],
                                         ZB[:, zoff+c:zoff+c+w],
                                         start=False, stop=True)
                        mux.eng().tensor_tensor(out=ZR[i][:, c:c+w],
                                                in0=ZR[i][:, c:c+w],
                                                in1=PU[:, 0:w],
                                                op=ALU.subtract)
                    if i == k + 1:
                        pending = list(_newton_cplx_steps(
                            nc, nwork, pmm, pmisc, ZR[i][:, 0:RW], consts,
                            NEWTON_Z))
                        drain(2)
                    else:
                        drain(2)
                drain(len(pending))

        # ---------------- P3: back-substitution ----------------
        with (
            tc.tile_pool(name="bs_work", bufs=3) as work,
            tc.tile_pool(name="bs_pacc", bufs=2, space="PSUM") as pacc,
            tc.tile_pool(name="bs_pmm", bufs=2, space="PSUM") as pmm,
        ):
            for k in range(NB - 1, -1, -1):
                if k < NB - 1:
                    P1a = pacc.tile([128, RW], F32, tag="bs_p1")
                    P2a = pacc.tile([128, RW], F32, tag="bs_p2")
                    nc.tensor.matmul(P1a[:], consts["nIb"][:], BF[k][:],
                                     start=True, stop=False)
                    for j in range(k + 1, NB):
                        utr = work.tile([128, 128], BF16, tag="bs_utr")
                        uti = work.tile([128, 128], BF16, tag="bs_uti")
                        nc.sync.dma_start_transpose(
                            utr[:], scr["utdram"][128*k:128*(k+1),
                                                  RW*j:RW*j+128])
                        nc.sync.dma_start_transpose(
                            uti[:], scr["utdram"][128*k:128*(k+1),
                                                  RW*j+128:RW*j+256])
                        last = (j == NB - 1)
                        nc.tensor.matmul(P1a[:], utr[:], BF[j][:],
                                         start=False, stop=last)
                        nc.tensor.matmul(P2a[:], uti[:], BF[j][:],
                                         start=(j == k + 1), stop=last)
                    W = work.tile([128, RW], BF16, tag="bs_W")
                    nc.vector.tensor_tensor(out=W[:, 0:128],
                                            in0=P2a[:, 128:256],
                                            in1=P1a[:, 0:128],
                                            op=ALU.subtract)
                    nc.gpsimd.scalar_tensor_tensor(
                        out=W[:, 128:256], in0=P1a[:, 128:256], scalar=-1.0,
                        in1=P2a[:, 0:128], op0=ALU.mult, op1=ALU.subtract)
                else:
                    W = BF[k]
                WB = work.tile([128, RW], BF16, tag="bs_WB")
                nc.vector.tensor_scalar(out=WB[:, 0:128], in0=W[:, 128:256],
                                        scalar1=-1.0, scalar2=None,
                                        op0=ALU.mult)
                nc.gpsimd.tensor_copy(WB[:, 128:256], W[:, 0:128])
                PS = pmm.tile([128, RW], F32, tag="bs_PS")
                nc.tensor.matmul(PS[:], ZR[k][:, 0:128], W[:],
                                 start=True, stop=False)
                nc.tensor.matmul(PS[:], ZR[k][:, 128:256], WB[:],
                                 start=False, stop=True)
                mux.eng().tensor_copy(BF[k][:], PS[:])
                nc.sync.dma_start(xdbg[128*k:128*(k+1), :], BF[k][:])

    # ---------------- P4: total field, power model, weights ----------------
    late = ctx.enter_context(tc.tile_pool(name="late", bufs=1))
    dvec = late.tile([128, LB], F32)
    drep = late.tile([128, L16], F32)
    wrep_r = late.tile([128, L16], F32)
    wrep_i = late.tile([128, L16], F32)
    vsum = late.tile([128, 2 * NB], F32)
    lam = late.tile([128, 1], F32)
    st_ = late.tile([128, LB], F32)
    srep = late.tile([128, LPAD], F32)
    yrep = late.tile([128, LPAD], F32)
    with (
        tc.tile_pool(name="p4_work", bufs=2) as work,
        tc.tile_pool(name="p4_pacc", bufs=1, space="PSUM") as pacc,
        tc.tile_pool(name="p4_pmisc", bufs=1, space="PSUM") as pmisc,
    ):
        Ptf1 = pacc.tile([40, RW], F32, tag="tf_p1")
        Ptf2 = pacc.tile([40, RW], F32, tag="tf_p2")
        for i in range(NB):
            gt = work.tile([128, 80], F32, tag="tf_g")
            nc.sync.dma_start(gt[:], din["gscT"][128*i:128*(i+1), :])
            gtb = work.tile([128, 80], BF16, tag="tf_gb")
            mux.eng().tensor_copy(gtb[:], gt[:])
            stt = (i == 0); spp = (i == NB - 1)
            nc.tensor.matmul(Ptf1[:], gtb[:, 0:40], BF[i][:],
                             start=stt, stop=spp)
            nc.tensor.matmul(Ptf2[:], gtb[:, 40:80], BF[i][:],
                             start=stt, stop=spp)
        df = work.tile([40, 80], F32, tag="tf_df")
        nc.sync.dma_start(df[:], din["dfpack"][:])
        tfr = work.tile([40, 40], F32, tag="tfr")
        tfi = work.tile([40, 40], F32, tag="tfi")
        nc.vector.tensor_tensor(out=tfr[:], in0=df[:, 0:40],
                                in1=Ptf1[:, 0:40], op=ALU.add)
        nc.vector.tensor_tensor(out=tfr[:], in0=tfr[:],
                                in1=Ptf2[:, 128:168], op=ALU.subtract)
        nc.gpsimd.tensor_tensor(out=tfi[:], in0=df[:, 40:80],
                                in1=Ptf1[:, 128:168], op=ALU.add)
        nc.gpsimd.tensor_tensor(out=tfi[:], in0=tfi[:],
                                in1=Ptf2[:, 0:40], op=ALU.add)
        tfd = work.tile([40, 80], F32, tag="tf_out")
        nc.vector.tensor_copy(tfd[:, 0:40], tfr[:])
        nc.vector.tensor_copy(tfd[:, 40:80], tfi[:])
        nc.sync.dma_start(tfdbg[:], tfd[:])

        mask40 = work.tile([40, 40], F32, tag="mask40")
        zero40 = work.tile([40, 1], F32, tag="zero40")
        nc.vector.memset(mask40[:], 1.0)
        nc.vector.memset(zero40[:], 0.0)
        nc.vector.copy_predicated(mask40[:], idu_s[0:40, 0:40],
                                  zero40[:].broadcast_to([40, 40]))

        pw = work.tile([40, 40], F32, tag="pw")
        nc.vector.tensor_tensor(out=pw[:], in0=tfr[:], in1=tfr[:], op=ALU.mult)
        t1 = work.tile([40, 40], F32, tag="pw_t")
        nc.gpsimd.tensor_tensor(out=t1[:], in0=tfi[:], in1=tfi[:], op=ALU.mult)
        nc.vector.tensor_tensor(out=pw[:], in0=pw[:], in1=t1[:], op=ALU.add)
        amp = work.tile([40, 40], F32, tag="amp")
        nc.scalar.activation(amp[:], pw[:], AF.Sqrt)
        nc.vector.tensor_scalar(out=amp[:], in0=amp[:], scalar1=NOISE,
                                scalar2=None, op0=ALU.add)
        nc.scalar.activation(amp[:], amp[:], AF.Ln)
        tpi = work.tile([40, 40], F32, tag="tpi")
        nc.vector.tensor_scalar(out=tpi[:], in0=amp[:], scalar1=C20L,
                                scalar2=CADD, op0=ALU.mult, op1=ALU.add)
        rec = work.tile([40, 40], F32, tag="rec")
        nc.vector.reciprocal(rec[:], pw[:])
        wr = work.tile([40, 40], F32, tag="wr")
        nc.vector.scalar_tensor_tensor(out=wr[:], in0=tfr[:], scalar=SA,
                                       in1=rec[:], op0=ALU.mult, op1=ALU.mult)
        nc.vector.tensor_tensor(out=wr[:], in0=wr[:], in1=mask40[:],
                                op=ALU.mult)
        wi = work.tile([40, 40], F32, tag="wi")
        nc.gpsimd.scalar_tensor_tensor(out=wi[:], in0=tfi[:], scalar=-SA,
                                       in1=rec[:], op0=ALU.mult, op1=ALU.mult)
        nc.gpsimd.tensor_tensor(out=wi[:], in0=wi[:], in1=mask40[:],
                                op=ALU.mult)
        tp40 = work.tile([40, 40], F32, tag="tp40")
        nc.sync.dma_start(tp40[:], din["tp40"][:])
        d40 = work.tile([40, 40], F32, tag="d40")
        nc.vector.tensor_tensor(out=d40[:], in0=tp40[:], in1=tpi[:],
                                op=ALU.subtract)
        nc.vector.scalar_tensor_tensor(out=d40[:], in0=d40[:],
                                       scalar=1.0 / LOG10E20, in1=mask40[:],
                                       op0=ALU.mult, op1=ALU.mult)

        def t40_store(src, dram, name):
            pt = pmisc.tile([40, 40], F32, tag=f"t40p_{name}")
            nc.tensor.matmul(pt[:], src[:], id_s[0:40, 0:40], start=True,
                             stop=True)
            d = work.tile([40, 40], F32, tag=f"t40_{name}")
            nc.vector.tensor_copy(d[:], pt[:])
            nc.sync.dma_start(bass.AP(dram, 0, [[1, L16]]), d[:])

        t40_store(d40, scr["sdram"], "d")
        t40_store(wr, scr["wrdram"], "wr")
        t40_store(wi, scr["widram"], "wi")

        row = work.tile([1, L16], F32, tag="rowld")
        nc.sync.dma_start(row[:], bass.AP(scr["sdram"], 0, [[1, L16]]))
        nc.gpsimd.partition_broadcast(drep[:], row[:])
        row = work.tile([1, L16], F32, tag="rowld")
        nc.sync.dma_start(row[:], bass.AP(scr["wrdram"], 0, [[1, L16]]))
        nc.gpsimd.partition_broadcast(wrep_r[:], row[:])
        row = work.tile([1, L16], F32, tag="rowld")
        nc.sync.dma_start(row[:], bass.AP(scr["widram"], 0, [[1, L16]]))
        nc.gpsimd.partition_broadcast(wrep_i[:], row[:])

        nc.vector.memset(dvec[:], 0.0)
        nc.sync.dma_start(dvec[:, 0:12],
                          bass.AP(scr["sdram"], 0, [[1, 128], [128, 12]]))
        nc.sync.dma_start(dvec[0:64, 12:13],
                          bass.AP(scr["sdram"], 1536, [[1, 64]]))

    # ---------------- P5+P7: H build fused with Gram quads ----------------
    ga_pool = ctx.enter_context(tc.tile_pool(name="ga", bufs=1))
    GA = [ga_pool.tile([128, LPAD], F32, tag=f"ga{l}", name=f"ga{l}")
          for l in range(LB)]
    with (
        tc.tile_pool(name="p5_hq", bufs=1) as hqpool,
        tc.tile_pool(name="p5_work", bufs=2) as work,
        tc.tile_pool(name="p5_pg", bufs=4, space="PSUM") as pg,
    ):
        HQ = [hqpool.tile([128, LPAD], BF16, tag=f"hq{s}", name=f"hq{s}")
              for s in range(4)]
        for s in range(4):
            nc.gpsimd.memset(HQ[s][:, L16:LPAD], 0.0)
        nc.vector.memset(vsum[:], 0.0)

        def gram_quad(q):
            for l in range(LB):
                c0 = 128 * l
                for cc in range(c0, LPAD, 512):
                    cw = min(512, LPAD - cc)
                    pgt = pg.tile([128, 512], F32, tag="g_pg")
                    for m in range(4):
                        nc.tensor.matmul(pgt[:, 0:cw], HQ[m][:, c0:c0+128],
                                         HQ[m][:, cc:cc+cw],
                                         start=(m == 0), stop=(m == 3))
                    if q == 0:
                        mux.eng().tensor_copy(GA[l][:, cc:cc+cw], pgt[:, 0:cw])
                    else:
                        mux.eng().tensor_tensor(out=GA[l][:, cc:cc+cw],
                                                in0=GA[l][:, cc:cc+cw],
                                                in1=pgt[:, 0:cw], op=ALU.add)

        for i in range(NB):
            sre = HQ[2 * (i % 2)]
            sim = HQ[2 * (i % 2) + 1]
            Gq = work.tile([128, 80], F32, tag="h_gq")
            Iq = work.tile([128, 80], F32, tag="h_iq")
            f_ap = fsc_s[:, i:i+1]
            nc.vector.tensor_scalar(out=Gq[:, 0:40], in0=BF[i][:, 168:208],
                                    scalar1=f_ap, scalar2=-1.0, op0=ALU.mult,
                                    op1=ALU.mult)
            nc.gpsimd.tensor_scalar(out=Gq[:, 40:80], in0=BF[i][:, 40:80],
                                    scalar1=f_ap, scalar2=None, op0=ALU.mult)
            nc.vector.tensor_scalar(out=Iq[:, 0:40], in0=BF[i][:, 128:168],
                                    scalar1=f_ap, scalar2=-1.0, op0=ALU.mult,
                                    op1=ALU.mult)
            nc.gpsimd.tensor_scalar(out=Iq[:, 40:80], in0=BF[i][:, 0:40],
                                    scalar1=f_ap, scalar2=None, op0=ALU.mult)
            GR3 = Gq[:, 0:40].rearrange("p (o r) -> p o r", o=1
                                        ).broadcast_to([128, 40, 40])
            GI3 = Gq[:, 40:80].rearrange("p (o r) -> p o r", o=1
                                         ).broadcast_to([128, 40, 40])
            IR3 = Iq[:, 0:40].rearrange("p (t o) -> p t o", o=1
                                        ).broadcast_to([128, 40, 40])
            II3 = Iq[:, 40:80].rearrange("p (t o) -> p t o", o=1
                                         ).broadcast_to([128, 40, 40])
            qr = work.tile([128, L16], F32, tag="h_qr")
            qi = work.tile([128, L16], F32, tag="h_qi")
            ta = work.tile([128, L16], F32, tag="h_ta")
            tb = work.tile([128, L16], F32, tag="h_tb")
            qr3 = qr[:].rearrange("p (t r) -> p t r", t=40)
            qi3 = qi[:].rearrange("p (t r) -> p t r", t=40)
            ta3 = ta[:].rearrange("p (t r) -> p t r", t=40)
            tb3 = tb[:].rearrange("p (t r) -> p t r", t=40)
            nc.vector.tensor_tensor(out=qr3, in0=GR3, in1=IR3, op=ALU.mult)
            nc.gpsimd.tensor_tensor(out=ta3, in0=GI3, in1=II3, op=ALU.mult)
            nc.gpsimd.tensor_tensor(out=qi3, in0=GI3, in1=IR3, op=ALU.mult)
            nc.vector.tensor_tensor(out=tb3, in0=GR3, in1=II3, op=ALU.mult)
            nc.vector.tensor_tensor(out=qr[:], in0=qr[:], in1=ta[:],
                                    op=ALU.subtract)
            nc.gpsimd.tensor_tensor(out=qi[:], in0=qi[:], in1=tb[:],
                                    op=ALU.add)
            # H rows: hr = qr*wr - qi*wi ; hi_stored = -(qr*wi + qi*wr)
            nc.vector.tensor_tensor(out=ta[:], in0=qr[:], in1=wrep_r[:],
                                    op=ALU.mult)
            nc.gpsimd.tensor_tensor(out=tb[:], in0=qi[:], in1=wrep_i[:],
                                    op=ALU.mult)
            nc.vector.tensor_tensor(out=sre[:, 0:L16], in0=ta[:], in1=tb[:],
                                    op=ALU.subtract)
            nc.gpsimd.tensor_tensor(out=ta[:], in0=qr[:], in1=wrep_i[:],
                                    op=ALU.mult)
            nc.vector.tensor_tensor(out=tb[:], in0=qi[:], in1=wrep_r[:],
                                    op=ALU.mult)
            nc.gpsimd.scalar_tensor_tensor(out=sim[:, 0:L16], in0=ta[:],
                                           scalar=-1.0, in1=tb[:],
                                           op0=ALU.mult, op1=ALU.subtract)
            junk = work.tile([128, L16], BF16, tag="h_junk")
            nc.vector.tensor_tensor_reduce(
                out=junk[:], in0=sre[:, 0:L16], in1=drep[:], scale=1.0,
                scalar=0.0, op0=ALU.mult, op1=ALU.add,
                accum_out=vsum[:, i:i+1])
            junk2 = work.tile([128, L16], BF16, tag="h_junk2")
            nc.gpsimd.scalar_tensor_tensor(
                out=junk2[:], in0=sim[:, 0:L16], scalar=1.0, in1=drep[:],
                op0=ALU.mult, op1=ALU.mult, accum_out=vsum[:, NB+i:NB+i+1])
            nc.sync.dma_start(scr["htdram"][128*i:128*(i+1), :], sre[:])
            nc.sync.dma_start(scr["htdram"][N+128*i:N+128*(i+1), :], sim[:])
            if i % 2 == 1:
                gram_quad(i // 2)

        vsq = work.tile([128, 2 * NB], F32, tag="vsq")
        nc.vector.tensor_tensor(out=vsq[:], in0=vsum[:], in1=vsum[:],
                                op=ALU.mult)
        vred = work.tile([128, 1], F32, tag="vred")
        nc.vector.tensor_reduce(vred[:], vsq[:], axis=AXX, op=ALU.add)
        nc.gpsimd.partition_all_reduce(vred[:], vred[:], 128,
                                       bass_isa.ReduceOp.add)
        nc.scalar.activation(lam[:], vred[:], AF.Sqrt)
        nc.vector.tensor_scalar(out=lam[:], in0=lam[:], scalar1=float(alpha),
                                scalar2=None, op0=ALU.mult)

    # ---------------- P8: scaled SPD block solve ----------------
    gr_pool = ctx.enter_context(tc.tile_pool(name="gr", bufs=1))
    GR = [gr_pool.tile([128, (LB - i) * 128], BF16, tag=f"gr{i}",
                       name=f"gr{i}") for i in range(LB)]
    BF2 = [gr_pool.tile([128, 1], BF16, tag=f"b2_{l}", name=f"b2_{l}")
           for l in range(LB)]
    ys = late.tile([128, LB], F32)
    with (
        tc.tile_pool(name="s_work", bufs=2) as work,
        tc.tile_pool(name="s_nw", bufs=1) as nwork,
        tc.tile_pool(name="s_pmm", bufs=1, space="PSUM") as pmm,
        tc.tile_pool(name="s_pup", bufs=1, space="PSUM") as pup,
        tc.tile_pool(name="s_pmisc", bufs=1, space="PSUM") as pmisc,
    ):
        # jacobi scaling vector from Gram diagonal
        gdiag = work.tile([128, LB], F32, tag="gdiag")
        for l in range(LB):
            t128 = work.tile([128, 128], F32, tag="gd_t")
            nc.gpsimd.scalar_tensor_tensor(
                out=t128[:], in0=GA[l][:, 128*l:128*(l+1)], scalar=1.0,
                in1=id_s[:], op0=ALU.mult, op1=ALU.mult,
                accum_out=gdiag[:, l:l+1])
        nc.vector.tensor_scalar(out=gdiag[:], in0=gdiag[:], scalar1=lam[:],
                                scalar2=None, op0=ALU.add)
        nc.scalar.activation(st_[:], gdiag[:], AF.Sqrt)
        nc.vector.reciprocal(st_[:], st_[:])
        ps_ = pmisc.tile([LB, 128], F32, tag="s_ps")
        nc.tensor.matmul(ps_[:], st_[:], id_s[:], start=True, stop=True)
        s13 = work.tile([LB, 128], F32, tag="s13")
        nc.vector.tensor_copy(s13[:], ps_[:])
        nc.sync.dma_start(bass.AP(scr["srowdram"], 0, [[1, LPAD]]), s13[:])
        srow = work.tile([1, LPAD], F32, tag="srow")
        nc.sync.dma_start(srow[:], bass.AP(scr["srowdram"], 0, [[1, LPAD]]))
        nc.gpsimd.partition_broadcast(srep[:], srow[:])

        onesb = work.tile([128, 1], BF16, tag="onesb")
        nc.vector.memset(onesb[:], 1.0)
        for i in range(LB):
            for j in range(i, LB):
                mux.eng().scalar_tensor_tensor(
                    out=GR[i][:, (j-i)*128:(j-i)*128+128],
                    in0=GA[i][:, 128*j:128*(j+1)], scalar=st_[:, i:i+1],
                    in1=srep[:, 128*j:128*(j+1)], op0=ALU.mult, op1=ALU.mult)
            nc.vector.copy_predicated(GR[i][:, 0:128], idu_s[:],
                                      onesb[:].broadcast_to([128, 128]))
        dsc = work.tile([128, LB], F32, tag="dsc")
        nc.vector.tensor_tensor(out=dsc[:], in0=dvec[:], in1=st_[:],
                                op=ALU.mult)
        for l in range(LB):
            nc.gpsimd.tensor_copy(BF2[l][:], dsc[:, l:l+1])

        pending = []

        def drain(n):
            for _ in range(min(n, len(pending))):
                pending.pop(0)()

        for step in _newton_real_steps(nc, nwork, pmm, pmisc, GR[0][:, 0:128],
                                       consts, NEWTON_SPD):
            step()
        for k in range(LB):
            nr = LB - 1 - k
            if nr > 0:
                # LT row = V_k @ (pivot row right of diag), wide
                LTrow = work.tile([128, (LB - 1) * 128], BF16, tag="lt_row")
                Wk = nr * 128
                for c in range(0, Wk, 512):
                    w = min(512, Wk - c)
                    pl = pup.tile([128, 512], F32, tag="s_pl")
                    nc.tensor.matmul(pl[:, 0:w], GR[k][:, 0:128],
                                     GR[k][:, 128+c:128+c+w],
                                     start=True, stop=True)
                    mux.eng().tensor_copy(LTrow[:, c:c+w], pl[:, 0:w])
            for i in range(k + 1, LB):
                lt = LTrow[:, (i-k-1)*128:(i-k)*128]
                pb = pmm.tile([128, 1], F32, tag="s_pb")
                nc.tensor.matmul(pb[:], lt, BF2[k][:], start=True, stop=True)
                nc.vector.tensor_tensor(out=BF2[i][:], in0=BF2[i][:],
                                        in1=pb[:], op=ALU.subtract)
                Wi = (LB - i) * 128
                for c in range(0, Wi, 512):
                    w = min(512, Wi - c)
                    pu = pup.tile([128, 512], F32, tag="s_pu")
                    nc.tensor.matmul(pu[:, 0:w], lt,
                                     GR[k][:, (i-k)*128+c:(i-k)*128+c+w],
                                     start=True, stop=True)
                    mux.eng().tensor_tensor(out=GR[i][:, c:c+w],
                                            in0=GR[i][:, c:c+w],
                                            in1=pu[:, 0:w], op=ALU.subtract)
                if i == k + 1:
                    pending = list(_newton_real_steps(
                        nc, nwork, pmm, pmisc, GR[i][:, 0:128], consts,
                        NEWTON_SPD))
                    drain(3)
                else:
                    drain(3)
            drain(len(pending))

        # backward substitution
        for k in range(LB - 1, -1, -1):
            P1a = pmm.tile([128, 1], F32, tag="s_pb")
            nc.tensor.matmul(P1a[:], consts["nIb"][:], BF2[k][:],
                             start=True, stop=(k == LB - 1))
            for j in range(k + 1, LB):
                utt = work.tile([128, 128], BF16, tag="s_utt")
                nc.sync.dma_start_transpose(
                    utt[:], GR[k][:, (j-k)*128:(j-k+1)*128])
                nc.tensor.matmul(P1a[:], utt[:], BF2[j][:],
                                 start=False, stop=(j == LB - 1))
            W2 = work.tile([128, 1], BF16, tag="s_W2")
            nc.vector.tensor_copy(W2[:], P1a[:])
            PS = pmm.tile([128, 1], F32, tag="s_pb")
            nc.tensor.matmul(PS[:], GR[k][:, 0:128], W2[:],
                             start=True, stop=True)
            nc.vector.tensor_scalar(out=BF2[k][:], in0=PS[:], scalar1=-1.0,
                                    scalar2=None, op0=ALU.mult)
        for l in range(LB):
            nc.gpsimd.tensor_copy(ys[:, l:l+1], BF2[l][:])
        nc.vector.tensor_tensor(out=ys[:], in0=ys[:], in1=st_[:], op=ALU.mult)
        psy = pmisc.tile([LB, 128], F32, tag="y_ps")
        nc.tensor.matmul(psy[:], ys[:], id_s[:], start=True, stop=True)
        y13 = work.tile([LB, 128], F32, tag="y13")
        nc.vector.tensor_copy(y13[:], psy[:])
        nc.sync.dma_start(bass.AP(scr["yrowdram"], 0, [[1, LPAD]]), y13[:])
        yrow = work.tile([1, LPAD], F32, tag="yrow")
        nc.sync.dma_start(yrow[:], bass.AP(scr["yrowdram"], 0, [[1, LPAD]]))
        nc.gpsimd.partition_broadcast(yrep[:], yrow[:])

    # ---------------- P9: chi = Ht y ----------------
    with tc.tile_pool(name="p9_work", bufs=3) as work:
        chi = late.tile([128, 2 * NB], F32)
        for ch in range(2 * NB):
            htc = work.tile([128, LPAD], BF16, tag="c_htc")
            nc.sync.dma_start(htc[:], scr["htdram"][128*ch:128*(ch+1), :])
            junk = work.tile([128, LPAD], BF16, tag="c_junk")
            mux.eng().scalar_tensor_tensor(
                out=junk[:], in0=htc[:], scalar=1.0, in1=yrep[:],
                op0=ALU.mult, op1=ALU.mult, accum_out=chi[:, ch:ch+1])
        nc.sync.dma_start(bass.AP(out_chi, 0, [[1, 128], [128, 2 * NB]]),
                          chi[:])
    ctx.close()


_CACHED = {}


def kernel(epsilon_r_iter, chi_iter, total_power, alpha, grid_x, grid_y,
           direct_field, incident_field, G_freespace, G_freespace_scaled,
           sensor_links):
    eps = np.asarray(epsilon_r_iter)
    chi_it = np.asarray(chi_iter)
    tp = np.asarray(total_power, dtype=np.float32)
    alpha_f = float(np.asarray(alpha))
    gx = np.asarray(grid_x, dtype=np.float32)
    gy = np.asarray(grid_y, dtype=np.float32)
    df = np.asarray(direct_field)
    einc = np.asarray(incident_field)
    gfs = np.asarray(G_freespace)
    gsc = np.asarray(G_freespace_scaled)
    links = np.asarray(sensor_links)

    # this kernel assumes the canonical uniform link set (t-major, r != t)
    expect = np.array([[t, r] for t in range(TX) for r in range(RX) if r != t],
                      dtype=np.int32)
    assert links.shape == expect.shape and np.array_equal(links, expect), \
        "kernel specialized for the canonical sensor_links layout"

    x = gx.T.reshape(N).astype(np.float32)
    y = gy.T.reshape(N).astype(np.float32)
    scat = np.real(eps.T.reshape(N)).astype(np.float32)

    geomS = np.stack([np.ones(N, np.float32), -2.0*x, -2.0*y,
                      (x*x + y*y)]).astype(np.float32)
    geomR = np.stack([(x*x + y*y), x, y,
                      np.ones(N, np.float32)]).astype(np.float32)
    scat_t = scat.reshape(NB, 128).T.copy()

    bpack = np.zeros((N, RW), np.float32)
    bpack[:, 0:40] = -einc.real; bpack[:, 40:80] = -gfs.real
    bpack[:, 128:168] = -einc.imag; bpack[:, 168:208] = -gfs.imag
    gscT = np.concatenate([gsc.real.T, gsc.imag.T], axis=1).astype(np.float32)
    dfpack = np.concatenate([df.real, df.imag], axis=1).astype(np.float32)

    # total_power [RX-1, TX] -> [40, 40] with zeros on the diagonal
    tp40 = np.zeros((40, 40), np.float32)
    for t in range(TX):
        rs = [r for r in range(RX) if r != t]
        tp40[rs, t] = tp[:, t]

    key = alpha_f
    if key not in _CACHED:
        _CACHED[key] = build_program(alpha_f)
    nc = _CACHED[key]

    id128 = np.eye(128, dtype=np.float32)
    im = {
        "geomS": geomS, "geomR": geomR, "scat_t": scat_t, "bpack": bpack,
        "gscT": gscT, "dfpack": dfpack, "tp40": tp40,
        "id128": id128, "idu8": id128.astype(np.uint8),
    }
    import os as _os
    _tr = _os.environ.get("KTRACE", "0") == "1"
    res = run_bass_kernel_spmd(nc, [im] * 8, core_ids=list(range(8)),
                               trace=_tr)
    out = res.results[0]
    _CACHED["last"] = (res, out)

    chi = np.asarray(out["out_chi"], dtype=np.float32)
    dchi_r = chi[:N].reshape(M, M).T
    dchi_i = chi[N:].reshape(M, M).T
    chi_new = (chi_it + (dchi_r + 1j * dchi_i)).astype(np.complex64)
    return chi_new + 1.0, chi_new
